# revision 32
# baseline (speedup 1.0000x reference)
"""Trainium2 Bass kernel: ragged mean-pool over [1, len_i] + Linear->tanh->Linear head.

Strategy (pure data parallel over batch, 8 NeuronCores):
  * Host: balance the 256 samples across 8 cores (32 each) by total row count,
    gather only the needed rows hidden_states[b, 1:len_b+1, :] into a packed
    dense array per core (the ragged/masked structure becomes a small 0/1
    "membership" matrix), and encode rows compactly (fp16 by default).
  * Device: stream packed row-tiles [128, 8*768]; for each 128-row subtile do
    pooled[b,h] += member[r,b] * rows[r,h] as a PE matmul with the membership
    matrix as the stationary operand, accumulating all tiles into one PSUM
    region. Then scale by 1/len, transpose, and run the tiny 768x768 tanh head
    and 96x768 classifier fully on-chip. Output is logits^T [96, 32] per core.
  * Host: scatter per-core outputs back to the full [256, 96] logits.

The compiled program depends only on (T_g, mode) where T_g = number of
1024-row groups per core -- not on the actual lengths -- so recompiles are
rare. All raggedness lives in data (packing + membership).
"""

import os
from contextlib import ExitStack

import numpy as np
import ml_dtypes

import concourse.bass as bass
import concourse.mybir as mybir
from concourse import bacc, bass_utils, tile

B, S, H, T_OUT = 256, 512, 768, 96
N_CORES = 8
LOCAL_B = B // N_CORES        # 32 samples per core
G = 8                         # packed rows per partition per group tile
ROWS_PER_GROUP = 128 * G      # 1024
F32 = mybir.dt.float32

# Row encodings: "f8" (1B/elem e4m3 + per-sample error-feedback + one mop-up
# residual row, ~6e-4 rel err, DoubleRow double-pumped PE), "f16" (2B/elem,
# ~2e-4 rel err), "f32x2" (bf16 hi+lo pair, 4B/elem, ~2e-6 rel err), "bf16"
# (2B/elem, ~1.4e-3 rel err).
MODE = os.environ.get("KERNEL_MODE", "f8")
# f8 pooling matmul width: 1 = double-wide DoubleRow (rhs free 1024, 2 matmuls
# per subtile pair), 0 = 3 matmuls of 512. Settable per-call via kernel.WIDE.
WIDE = int(os.environ.get("KERNEL_WIDE", "1"))
# f8 pooled-scale split: 1 = DVE [0:256] + ACT [256:512],[512:768] (transpose
# chain starts ~0.4us earlier), 0 = DVE [0:512] + ACT [512:768].
SC3 = int(os.environ.get("KERNEL_SC3", "0"))
# f8 stream tail chunking: 1 = taper the last chunks ([6,4,2] subtiles), 0 =
# uniform 8-subtile chunks. Settable per-call via kernel.TAPER.
TAPER = int(os.environ.get("KERNEL_TAPER", "0"))
# "raw" = hand-synchronized Bacc program (no Tile scheduler, minimal
# semaphore traffic and no kernel-tail sem-reset butterfly); "tile" = the
# TileContext-scheduled variant.
IMPL = os.environ.get("KERNEL_IMPL", "raw")

_cache: dict = {}
last_results = None  # BassKernelResults of the most recent run (for test.py)


def _build_program(T_g: int, mode: str) -> bass.Bass:
    sdt = mybir.dt.float16 if mode == "f16" else mybir.dt.bfloat16
    n_streams = 2 if mode == "f32x2" else 1
    W = G * H  # free-dim width of a group tile

    # Bacc (not raw Bass): its compile() pass splits multi-semaphore waits
    # into EventSemaphore chains — hardware allows at most 1 wait per
    # instruction — and moves matmul waits onto ldweights.
    nc = bacc.Bacc()
    streams = [
        nc.declare_dram_parameter(f"hs{i}", [T_g, 128, W], sdt, isOutput=False)
        for i in range(n_streams)
    ]
    member = nc.declare_dram_parameter(
        "member", [128, T_g * G * LOCAL_B], sdt, isOutput=False
    )
    dwT = nc.declare_dram_parameter("dwT", [128, 6 * H], F32, isOutput=False)
    cwT = nc.declare_dram_parameter("cwT", [128, 6 * T_OUT], F32, isOutput=False)
    db6 = nc.declare_dram_parameter("db6", [128, 6], F32, isOutput=False)
    cb1 = nc.declare_dram_parameter("cb1", [T_OUT, 1], F32, isOutput=False)
    invl = nc.declare_dram_parameter("invl", [LOCAL_B, 1], F32, isOutput=False)
    ident = nc.declare_dram_parameter("ident", [32, 32], F32, isOutput=False)
    out = nc.declare_dram_parameter("out", [T_OUT, LOCAL_B], F32, isOutput=True)

    with ExitStack() as ctx:
        tc = ctx.enter_context(tile.TileContext(nc))
        const_pool = ctx.enter_context(tc.tile_pool(name="const", bufs=1))
        # All group tiles resident at once (single-stream modes fit: T_g * 12KB
        # per partition). Slot reuse would attach 3 semaphore waits to the
        # reload DMAs, which the DMA instruction encoding cannot carry.
        in_bufs = T_g * n_streams if n_streams == 1 else 3
        in_pool = ctx.enter_context(tc.tile_pool(name="inp", bufs=in_bufs))
        sb_pool = ctx.enter_context(tc.tile_pool(name="sb", bufs=1))
        ps_pooled = ctx.enter_context(tc.tile_pool(name="psp", bufs=1, space="PSUM"))
        ps_small = ctx.enter_context(tc.tile_pool(name="pss", bufs=2, space="PSUM"))

        # DMA order matters: queues drain in emission order, and the pooling
        # matmuls only need `member` + their stream tile. Load those first;
        # the head weights (dwT/cwT, ~2.7MB) are consumed only after all
        # pooling, so they stream in behind and overlap the pooling phase.
        member_t = const_pool.tile([128, T_g * G * LOCAL_B], sdt)
        nc.sync.dma_start(member_t[:], member[:])

        all_stiles = []
        for t in range(T_g):
            stiles = []
            for si, s in enumerate(streams):
                st = in_pool.tile([128, W], sdt, tag=f"s{si}")
                nc.sync.dma_start(st[:], s[t])
                stiles.append(st)
            all_stiles.append(stiles)

        invl_t = const_pool.tile([LOCAL_B, 1], F32)
        nc.sync.dma_start(invl_t[:], invl[:])
        ident_t = const_pool.tile([32, 32], F32)
        nc.sync.dma_start(ident_t[:], ident[:])
        dwT_t = const_pool.tile([128, 6 * H], F32)
        nc.sync.dma_start(dwT_t[:], dwT[:])
        cwT_t = const_pool.tile([128, 6 * T_OUT], F32)
        nc.sync.dma_start(cwT_t[:], cwT[:])
        db6_t = const_pool.tile([128, 6], F32)
        nc.sync.dma_start(db6_t[:], db6[:])
        cb1_t = const_pool.tile([T_OUT, 1], F32)
        nc.sync.dma_start(cb1_t[:], cb1[:])

        # Pre-touch small const tiles on the engine that will consume them:
        # several ISA instruction encodings carry only ONE semaphore wait, so
        # the consuming op must not need both its data-producer wait and a
        # const-DMA wait. Touching the const here advances that engine's
        # observed clock past the const DMA, and the later wait is elided.
        scratch = const_pool.tile([128, 8], F32)
        nc.vector.tensor_copy(scratch[:LOCAL_B, 0:1], invl_t[:])
        nc.vector.tensor_copy(scratch[:T_OUT, 1:2], cb1_t[:])
        nc.scalar.activation(
            scratch[:, 2:8], db6_t[:], mybir.ActivationFunctionType.Copy
        )

        # ---- ragged pooling: pooled[b, h] = sum over packed rows r of
        #      member[r, b] * row[r, h], accumulated in PSUM over all tiles.
        pooled_a = ps_pooled.tile([LOCAL_B, 512], F32, tag="pa")
        pooled_b = ps_pooled.tile([LOCAL_B, H - 512], F32, tag="pb")
        n_mm = T_g * G * n_streams  # matmuls per PSUM region
        i_mm = 0
        for t in range(T_g):
            stiles = all_stiles[t]
            for q in range(G):
                k = t * G + q
                lhsT = member_t[:, k * LOCAL_B : (k + 1) * LOCAL_B]
                for st in stiles:
                    first, last = i_mm == 0, i_mm == n_mm - 1
                    nc.tensor.matmul(
                        pooled_a[:], lhsT, st[:, q * H : q * H + 512],
                        start=first, stop=last,
                    )
                    nc.tensor.matmul(
                        pooled_b[:], lhsT, st[:, q * H + 512 : (q + 1) * H],
                        start=first, stop=last,
                    )
                    i_mm += 1

        # ---- mean: scale each sample's partition by 1/len
        pooled_sb = sb_pool.tile([LOCAL_B, H], F32)
        nc.vector.tensor_scalar_mul(pooled_sb[:, 0:512], pooled_a[:], invl_t[:])
        nc.vector.tensor_scalar_mul(pooled_sb[:, 512:H], pooled_b[:], invl_t[:])

        # ---- transpose pooled [32, 768] -> pooledT [768, 32] via PE
        pooledT_sb = sb_pool.tile([128, 6 * LOCAL_B], F32)
        for c in range(6):
            tp = ps_small.tile([128, LOCAL_B], F32, tag="tp")
            nc.tensor.transpose(
                tp[:], pooled_sb[:, c * 128 : (c + 1) * 128], ident_t[:]
            )
            nc.vector.tensor_copy(pooledT_sb[:, c * LOCAL_B : (c + 1) * LOCAL_B], tp[:])

        # ---- dense layer + tanh: hT[j, b] = tanh(dense_b[j] + sum_h dwT[h, j] pooledT[h, b])
        hT_sb = sb_pool.tile([128, 6 * LOCAL_B], F32)
        for jg in range(6):
            hps = ps_small.tile([128, LOCAL_B], F32, tag="hps")
            for c in range(6):
                nc.tensor.matmul(
                    hps[:],
                    dwT_t[:, c * H + jg * 128 : c * H + (jg + 1) * 128],
                    pooledT_sb[:, c * LOCAL_B : (c + 1) * LOCAL_B],
                    start=(c == 0), stop=(c == 5),
                )
            nc.scalar.activation(
                hT_sb[:, jg * LOCAL_B : (jg + 1) * LOCAL_B],
                hps[:],
                mybir.ActivationFunctionType.Tanh,
                bias=db6_t[:, jg : jg + 1],
            )

        # ---- classifier: logitsT[t, b] = cls_b[t] + sum_j cwT[j, t] hT[j, b]
        lps = ps_small.tile([T_OUT, LOCAL_B], F32, tag="lps")
        for jg in range(6):
            nc.tensor.matmul(
                lps[:],
                cwT_t[:, jg * T_OUT : (jg + 1) * T_OUT],
                hT_sb[:, jg * LOCAL_B : (jg + 1) * LOCAL_B],
                start=(jg == 0), stop=(jg == 5),
            )
        logits_sb = sb_pool.tile([T_OUT, LOCAL_B], F32)
        nc.vector.tensor_scalar_add(logits_sb[:], lps[:], cb1_t[:])
        # SWDGE store: lands on a fresh DMASW sem lane, so it carries only the
        # DVE wait (every encoding has a single wait slot).
        nc.gpsimd.dma_start(out[:], logits_sb[:])

    nc.compile()
    return nc


F8 = mybir.dt.float8e4
P8 = 128                      # stream partitions in f8 mode (124 was tried to
                              # starve slow SDMA engine 15, but non-128
                              # partition DMAs fall off the fast descriptor
                              # path: 2x slower overall)
NP_F8 = ml_dtypes.float8_e4m3  # IEEE-style e4m3 (max 240) == TRN FP8_EXP4
F16 = mybir.dt.float16


def _build_program_f8(
    n_full: int, g_last: int, wide: bool = False, taper: bool = False,
    sc3: bool = False,
) -> bass.Bass:
    """fp8 variant: e4m3 streams + membership, DoubleRow double-pumped pooling
    matmuls (256-row contraction per instruction), fp16 head.

    Layout: one flat stream tensor sdata [128, K_cols, H] (K_cols 128-row
    subtiles; groups of 8 subtiles share a membership accumulation pattern,
    the optional last group holds g_last even subtiles). All padding rows are
    host-zeroed (fp8 0x00) so there are no device memsets and no gpsimd use.

    DMA: the stream rides the Sync HWDGE ring in a few LARGE chunks (small
    first chunk for a fast pipeline start) -- per-instruction descriptor
    generation (~0.7-1.3us) otherwise cannot keep 16 SDMA engines fed at
    ~400GB/s with halved (fp8) per-instruction bytes. Everything else
    (member, smalls, head weights) rides the Scalar/ACT HWDGE ring
    concurrently, so head weights arrive early without delaying the stream
    tail."""
    n_groups = n_full + (1 if g_last else 0)
    g_of = lambda t: 8 if t < n_full else g_last
    K_cols = 8 * n_full + g_last  # total 128-row subtiles
    DR = mybir.MatmulPerfMode.DoubleRow

    # stream chunk boundaries in subtile units: one chunk per group. Fine
    # granularity keeps the tail exposure small (only the last small chunk
    # gates the final pooling) while ~0.8us/instruction descriptor-gen still
    # stays well ahead of the ~1.9us/group transfer time.
    if taper:
        TAPS = {12: [6, 4, 2], 10: [4, 4, 2], 8: [4, 2, 2], 6: [4, 2],
                4: [2, 2], 2: [2]}
        bnds = [0]
        while K_cols - bnds[-1] > 12:
            bnds.append(bnds[-1] + 8)
        for step in TAPS.get(K_cols - bnds[-1], [K_cols - bnds[-1]]):
            bnds.append(bnds[-1] + step)
    else:
        bnds = [0]
        while bnds[-1] < K_cols:
            bnds.append(min(bnds[-1] + 8, K_cols))
    n_chunks = len(bnds) - 1
    chunk_of_sub = {}
    for i in range(n_chunks):
        for s in range(bnds[i], bnds[i + 1]):
            chunk_of_sub[s] = i

    nc = bacc.Bacc(enable_partition_id=False)
    hs_d = nc.declare_dram_parameter("hs", [P8, K_cols * H], F8, isOutput=False)
    member_d = nc.declare_dram_parameter(
        "member", [P8, K_cols * LOCAL_B], F8, isOutput=False
    )
    dwT_d = nc.declare_dram_parameter("dwT", [128, 6 * H], F16, isOutput=False)
    cwT_d = nc.declare_dram_parameter("cwT", [128, 6 * T_OUT], F16, isOutput=False)
    ident2_d = nc.declare_dram_parameter("ident2", [32, 32], F16, isOutput=False)
    # smalls blob [128, 8] f32: col 0 invl (rows 0-31), cols 2-7 dense_b chunks
    smalls_d = nc.declare_dram_parameter("smalls", [128, 8], F32, isOutput=False)
    clsb_d = nc.declare_dram_parameter("clsb", [LOCAL_B, T_OUT], F32, isOutput=False)
    out_d = nc.declare_dram_parameter("out", [LOCAL_B, T_OUT], F32, isOutput=True)

    with ExitStack() as ctx:
        member_t = ctx.enter_context(
            nc.sbuf_tensor([P8, K_cols, LOCAL_B], F8)
        )
        sdata = ctx.enter_context(nc.sbuf_tensor([P8, K_cols, H], F8))
        smalls_t = ctx.enter_context(nc.sbuf_tensor([128, 8], F32))
        ident2_t = ctx.enter_context(nc.sbuf_tensor([32, 32], F16))
        dwT_t = ctx.enter_context(nc.sbuf_tensor([128, 6 * H], F16))
        cwT_t = ctx.enter_context(nc.sbuf_tensor([128, 6 * T_OUT], F16))
        pooled_sb = ctx.enter_context(nc.sbuf_tensor([LOCAL_B, H], F16))
        pooledT_sb = ctx.enter_context(nc.sbuf_tensor([128, 6 * LOCAL_B], F16))
        hT_sb = ctx.enter_context(nc.sbuf_tensor([128, 6 * LOCAL_B], F16))
        clsb_t = ctx.enter_context(nc.sbuf_tensor([LOCAL_B, T_OUT], F32))
        logits_sb = ctx.enter_context(nc.sbuf_tensor([LOCAL_B, T_OUT], F32))

        pooled_a = ctx.enter_context(nc.psum_tensor([LOCAL_B, 512], F32))
        pooled_b = ctx.enter_context(nc.psum_tensor([LOCAL_B, 512], F32))
        tp = [
            ctx.enter_context(nc.psum_tensor(f"tp{i}", [128, 512], F16))
            for i in range(3)
        ]
        hps = [
            ctx.enter_context(nc.psum_tensor(f"hps{i}", [128, 512], F32))
            for i in range(2)
        ]
        lps = ctx.enter_context(nc.psum_tensor([LOCAL_B, 512], F32))

        invl_ap = smalls_t[:LOCAL_B, 0:1]
        db6_ap = smalls_t[:, 2:8]

        s_member = nc.alloc_semaphore("s_member")
        s_chunk = [nc.alloc_semaphore(f"s_chunk{i}") for i in range(n_chunks)]
        s_smalls = nc.alloc_semaphore("s_smalls")
        s_hw = nc.alloc_semaphore("s_hw")  # dwT+cwT (adjacent on one ring:
        # a full-count wait of 32 implies both transfers complete)
        s_pool = nc.alloc_semaphore("s_pool")
        s_scA = nc.alloc_semaphore("s_scA")
        s_scB = nc.alloc_semaphore("s_scB")
        s_tr = nc.alloc_semaphore("s_tr")
        s_ptcopy = nc.alloc_semaphore("s_ptcopy")
        s_head = nc.alloc_semaphore("s_head")
        s_tanh = nc.alloc_semaphore("s_tanh")
        s_cls = nc.alloc_semaphore("s_cls")
        s_log = nc.alloc_semaphore("s_log")
        s_out = nc.alloc_semaphore("s_out")

        with nc.Block() as block:

            @block.sync
            def _(sync):
                for i in range(n_chunks):
                    a, b = bnds[i], bnds[i + 1]
                    sync.dma_start(
                        out=sdata[:, a:b, :], in_=hs_d[:, a * H : b * H]
                    ).then_inc(s_chunk[i], 16)
                # Head weights ride the sync queue AFTER every stream chunk:
                # the queue drains FIFO per engine, so they never steal
                # bandwidth mid-stream (a mid-stream transfer delays a chunk
                # by ~4.5us, idles the PE past the HAM window, and the
                # re-throttled PE then runs the rest of the pooling at half
                # clock). They land ~3us after the last chunk, just before
                # the dense layer needs them.
                sync.dma_start(out=dwT_t[:], in_=dwT_d[:]).then_inc(s_hw, 16)
                sync.dma_start(out=cwT_t[:], in_=cwT_d[:]).then_inc(s_hw, 16)
                sync.wait_ge(s_log, 1)
                sync.dma_start(out=out_d[:], in_=logits_sb[:]).then_inc(s_out, 16)
                sync.wait_ge(s_out, 16)

            @block.scalar
            def _(scalar):
                # second HWDGE ring: member + small consts + head weights
                scalar.dma_start(out=member_t[:], in_=member_d[:]).then_inc(
                    s_member, 16
                )
                scalar.dma_start(out=smalls_t[:], in_=smalls_d[:]).then_inc(
                    s_smalls, 16
                )
                scalar.dma_start(out=ident2_t[:], in_=ident2_d[:]).then_inc(
                    s_smalls, 16
                )
                scalar.dma_start(out=clsb_t[:], in_=clsb_d[:]).then_inc(s_smalls, 16)
                scalar.wait_ge(s_smalls, 48)
                # dummy tanh: pulls the ~1.3us ACT_TABLE_LOAD off the critical
                # path. Overwritten by the real jg=0 tanh later.
                nc.scalar.activation(
                    hT_sb[:, 0:1], smalls_t[:, 0:1],
                    mybir.ActivationFunctionType.Tanh,
                ).then_inc(s_tanh, 1)
                # pooled scale, ACT side (concurrent with the DVE side)
                scalar.wait_ge(s_pool, 1)
                if sc3:
                    nc.scalar.activation(
                        pooled_sb[:, 256:512], pooled_a[:, 256:512],
                        mybir.ActivationFunctionType.Copy,
                        scale=invl_ap,
                    ).then_inc(s_scB, 1)
                nc.scalar.activation(
                    pooled_sb[:, 512:H], pooled_b[:, : H - 512],
                    mybir.ActivationFunctionType.Copy,
                    scale=invl_ap,
                ).then_inc(s_scB, 1)
                for jg in range(6):
                    scalar.wait_ge(s_head, jg + 1)
                    nc.scalar.activation(
                        hT_sb[:, jg * LOCAL_B : (jg + 1) * LOCAL_B],
                        hps[jg % 2][:, :LOCAL_B],
                        mybir.ActivationFunctionType.Tanh,
                        bias=db6_ap[:, jg : jg + 1],
                    ).then_inc(s_tanh, 1)

            @block.tensor
            def _(tensor):
                # HAM keep-warm fillers: matmuls on (possibly uninitialized)
                # stream bytes into the hps scratch bank; any NaNs land in
                # PSUM that the dense phase later resets with start=True.
                def filler(n):
                    for _ in range(n):
                        nc.tensor.matmul(
                            hps[0][:, :512],
                            sdata[:, 0:1, 0:128],
                            sdata[:, 0:1, 0:512],
                            start=True, stop=True,
                        )

                filler(12)
                tensor.wait_ge(s_member, 16)
                n_pairs = K_cols // 2
                i_mm = 0
                last_mm = None
                cur_chunk = -1
                for t in range(n_groups):
                    g = g_of(t)
                    k_off = 8 * t
                    for qp in range(0, g, 2):
                        need = chunk_of_sub[k_off + qp + 1]
                        if need > cur_chunk:
                            filler(2)
                            tensor.wait_ge(s_chunk[need], 16)
                            cur_chunk = need
                        first, last = i_mm == 0, i_mm == n_pairs - 1
                        if first:
                            # start=True zeroes PSUM at region (bank)
                            # granularity: open each bank with exactly ONE
                            # full-width start=True write (plain fp8 matmuls
                            # over the first subtile pair); everything after
                            # accumulates with start=False.
                            for sub in (0, 1):
                                nc.tensor.matmul(
                                    pooled_a[:, 0:512],
                                    member_t[:, k_off + sub : k_off + sub + 1, :],
                                    sdata[:, k_off + sub : k_off + sub + 1, 0:512],
                                    start=(sub == 0), stop=False,
                                )
                                last_mm = nc.tensor.matmul(
                                    pooled_b[:, 0:256],
                                    member_t[:, k_off + sub : k_off + sub + 1, :],
                                    sdata[:, k_off + sub : k_off + sub + 1, 512:768],
                                    start=(sub == 0), stop=last,
                                )
                        else:
                            lhsT = member_t[:, k_off + qp : k_off + qp + 2, :]
                            pair = sdata[:, k_off + qp : k_off + qp + 2, :]
                            if wide:
                                # rhs free 1024 (fp8 moving max per HW, above
                                # bass's unused fp32-era 512 constant): 2
                                # matmuls per pair, bank A one full region
                                nc.tensor.matmul(
                                    pooled_a[:, 0:512], lhsT, pair[:, :, 0:512],
                                    start=False, stop=last, perf_mode=DR,
                                )
                                last_mm = nc.tensor.matmul(
                                    pooled_b[:, 0:256], lhsT, pair[:, :, 512:768],
                                    start=False, stop=last, perf_mode=DR,
                                )
                            else:
                                for out_ap, h0 in (
                                    (pooled_a[:, 0:256], 0),
                                    (pooled_a[:, 256:512], 256),
                                    (pooled_b[:, 0:256], 512),
                                ):
                                    last_mm = nc.tensor.matmul(
                                        out_ap, lhsT,
                                        pair[:, :, h0 : h0 + 256],
                                        start=False, stop=last, perf_mode=DR,
                                    )
                        i_mm += 1
                last_mm.then_inc(s_pool, 1)
                # transposes (need the scales + the identity matrix)
                filler(2)
                tensor.wait_ge(s_smalls, 48)
                for c in range(6):
                    if sc3:
                        if c < 2:
                            tensor.wait_ge(s_scA, 1)
                        else:
                            tensor.wait_ge(s_scB, 1 if c < 4 else 2)
                    else:
                        tensor.wait_ge(s_scA if c < 4 else s_scB, 1)
                    if c >= 3:
                        tensor.wait_ge(s_ptcopy, c - 2)
                    nc.tensor.transpose(
                        tp[c % 3][:, :LOCAL_B],
                        pooled_sb[:, c * 128 : (c + 1) * 128],
                        ident2_t[:],
                    ).then_inc(s_tr, 1)
                # dense layer
                filler(2)
                tensor.wait_ge(s_ptcopy, 6)
                tensor.wait_ge(s_hw, 32)
                for jg in range(6):
                    if jg >= 2:
                        tensor.wait_ge(s_tanh, jg)
                    for c in range(6):
                        mm = nc.tensor.matmul(
                            hps[jg % 2][:, :LOCAL_B],
                            dwT_t[:, c * H + jg * 128 : c * H + (jg + 1) * 128],
                            pooledT_sb[:, c * LOCAL_B : (c + 1) * LOCAL_B],
                            start=(c == 0), stop=(c == 5),
                        )
                    mm.then_inc(s_head, 1)
                # classifier: logits[b, t] -- hT chunk is the stationary
                # operand so the output lands batch-major.
                for jg in range(6):
                    tensor.wait_ge(s_tanh, jg + 2)
                    mm = nc.tensor.matmul(
                        lps[:, :T_OUT],
                        hT_sb[:, jg * LOCAL_B : (jg + 1) * LOCAL_B],
                        cwT_t[:, jg * T_OUT : (jg + 1) * T_OUT],
                        start=(jg == 0), stop=(jg == 5),
                    )
                mm.then_inc(s_cls, 1)

            @block.vector
            def _(vector):
                vector.wait_ge(s_smalls, 48)
                vector.wait_ge(s_pool, 1)
                nc.vector.tensor_scalar_mul(
                    pooled_sb[:, 0 : 256 if sc3 else 512],
                    pooled_a[:, 0 : 256 if sc3 else 512], invl_ap
                ).then_inc(s_scA, 1)
                for c in range(6):
                    vector.wait_ge(s_tr, c + 1)
                    nc.vector.tensor_copy(
                        pooledT_sb[:, c * LOCAL_B : (c + 1) * LOCAL_B],
                        tp[c % 3][:, :LOCAL_B],
                    ).then_inc(s_ptcopy, 1)
                vector.wait_ge(s_cls, 1)
                nc.vector.tensor_add(
                    logits_sb[:], lps[:, :T_OUT], clsb_t[:]
                ).then_inc(s_log, 1)

    nc.compile()
    return nc


def _quantize_feedback_f8(hs, lens):
    """e4m3 quantization with per-(sample, channel) error feedback along the
    row sequence, plus a final mop-up row holding the residual. The device sums
    rows, so per-row quantization errors telescope: the pooled sum sees only
    the (quantized) final residual."""
    f32 = np.float32
    q = np.zeros((B, S, H), NP_F8)
    err = np.zeros((B, H), f32)
    maxlen = int(lens.max())
    hsf = hs.astype(f32, copy=False)
    for s in range(1, maxlen + 1):
        active = lens >= s
        x = hsf[active, s, :] + err[active]
        qs = x.astype(NP_F8)
        q[active, s, :] = qs
        err[active] = x - qs.astype(f32)
    mop = err.astype(NP_F8)
    return q, mop


def _kernel_f8(hs, lens, dense_w, dense_b, cls_w, cls_b):
    global last_results
    # ---- assign samples to cores: greedy LPT on (len+1) with a 32-per-core cap
    w = lens + 1  # +1 for the mop-up row
    order = np.argsort(-w, kind="stable")
    core_samples = [[] for _ in range(N_CORES)]
    load = np.zeros(N_CORES, dtype=np.int64)
    for b in order:
        open_cores = [c for c in range(N_CORES) if len(core_samples[c]) < LOCAL_B]
        c = min(open_cores, key=lambda c: load[c])
        core_samples[c].append(int(b))
        load[c] += int(w[b])
    max_rows = int(load.max())
    n_full, rem = divmod(max_rows, 8 * P8)
    g_last = -(-rem // P8)
    g_last += g_last % 2  # DoubleRow consumes subtile pairs
    if g_last == 8:
        n_full, g_last = n_full + 1, 0
    n_groups = n_full + (1 if g_last else 0)
    NR = 8 * P8 * n_full + P8 * g_last
    K_cols = 8 * n_full + g_last

    key = ("f8", n_full, g_last, WIDE, TAPER, SC3)
    if key not in _cache:
        _cache[key] = _build_program_f8(
            n_full, g_last, bool(WIDE), bool(TAPER), bool(SC3)
        )
    nc = _cache[key]

    q, mop = _quantize_feedback_f8(hs, lens)

    # ---- shared (replicated) head tensors, fp16
    dwT_host = np.empty((128, 6 * H), np.float32)
    for c in range(6):
        dwT_host[:, c * H : (c + 1) * H] = dense_w[:, c * 128 : (c + 1) * 128].T
    cwT_host = np.empty((128, 6 * T_OUT), np.float32)
    for jg in range(6):
        cwT_host[:, jg * T_OUT : (jg + 1) * T_OUT] = cls_w[:, jg * 128 : (jg + 1) * 128].T
    dwT_host = dwT_host.astype(np.float16)
    cwT_host = cwT_host.astype(np.float16)
    ident2_host = np.eye(32, dtype=np.float16)
    db6_host = np.ascontiguousarray(dense_b.reshape(6, 128).T)
    clsb_host = np.ascontiguousarray(
        np.broadcast_to(cls_b, (LOCAL_B, T_OUT)).astype(np.float32)
    )

    q2 = q.reshape(B * S, H)
    in_maps = []
    for c in range(N_CORES):
        samples = core_samples[c]
        lens_c = lens[samples]
        packed = np.zeros((NR, H), NP_F8)
        pos = 0
        for b in samples:
            L = int(lens[b])
            packed[pos : pos + L] = q2[b * S + 1 : b * S + 1 + L]
            packed[pos + L] = mop[b]
            pos += L + 1

        # membership: row j -> (group t, partition p, subtile q); G_t rows are
        # consecutive per partition within a group.
        j = np.arange(pos)
        t = np.minimum(j // (8 * P8), n_groups - 1)
        j2 = j - t * (8 * P8)
        g_t = np.where(t < n_full, 8, g_last)
        p = j2 // g_t
        qsub = j2 - p * g_t
        kcol = 8 * t + qsub
        local_b = np.repeat(np.arange(LOCAL_B), lens_c + 1)
        mem = np.zeros((P8, K_cols * LOCAL_B), NP_F8)
        mem[p, kcol * LOCAL_B + local_b] = NP_F8(1.0)

        smalls = np.zeros((128, 8), np.float32)
        smalls[:LOCAL_B, 0] = 1.0 / lens_c.astype(np.float32)
        smalls[:, 2:8] = db6_host
        im = {
            "member": mem,
            "dwT": dwT_host,
            "cwT": cwT_host,
            "ident2": ident2_host,
            "smalls": smalls,
            "clsb": clsb_host,
        }
        # sdata layout [P8, K_cols, H]: row j -> sdata[p_j, kcol_j, :]
        arr = np.zeros((P8, K_cols, H), NP_F8)
        arr[p, kcol] = packed[:pos]
        im["hs"] = arr.reshape(P8, K_cols * H)
        in_maps.append(im)

    trace = bool(os.environ.get("KERNEL_TRACE"))
    try:
        res = bass_utils.run_bass_kernel_spmd(
            nc, in_maps, list(range(N_CORES)), trace=trace
        )
    except Exception:
        res = bass_utils.run_bass_kernel_spmd(
            nc, in_maps, list(range(N_CORES)), trace=trace
        )
    last_results = res

    logits = np.zeros((B, T_OUT), np.float32)
    for c in range(N_CORES):
        logits[core_samples[c], :] = res.results[c]["out"]
    return logits


def _build_program_raw(T_g: int, mode: str, p_last: int = 128) -> bass.Bass:
    """Hand-synchronized variant: one FIFO HWDGE ring delivers member, the
    stream tiles (in consumption order), then the head weights; each engine's
    program carries explicit sem waits. PSUM is budgeted bank-by-bank:
    pooled_a, pooled_b, tp0-2, hps0-1, lps = 8 banks."""
    sdt = mybir.dt.float16 if mode == "f16" else mybir.dt.bfloat16
    n_streams = 2 if mode == "f32x2" else 1
    # Head dtype: fp16/bf16 single-stream modes run the whole head in the
    # stream dtype (fp32 head matmuls cost 2 LDWEIGHTS+MATMUL passes each —
    # measured ~17us for the 48 head matmuls vs ~5us in fp16). The f32x2
    # accuracy mode keeps the head in fp32.
    hdt = F32 if n_streams == 2 else sdt
    W = G * H

    # No collectives -> no partition id; skipping it drops 5 per-engine
    # TENSOR_LOADs (~2us) from the launch preamble.
    nc = bacc.Bacc(enable_partition_id=False)
    streams_d = [
        nc.declare_dram_parameter(f"hs{i}", [T_g, 128, W], sdt, isOutput=False)
        for i in range(n_streams)
    ]
    member_d = nc.declare_dram_parameter(
        "member", [128, T_g * G * LOCAL_B], sdt, isOutput=False
    )
    dwT_d = nc.declare_dram_parameter("dwT", [128, 6 * H], hdt, isOutput=False)
    cwT_d = nc.declare_dram_parameter("cwT", [128, 6 * T_OUT], hdt, isOutput=False)
    ident2_d = nc.declare_dram_parameter("ident2", [32, 32], hdt, isOutput=False)
    # smalls blob [128, 40] f32: col 0 invl (rows 0-31), col 1 cls_b (rows
    # 0-95), cols 2-7 dense_b chunks, cols 8-39 identity (rows 0-31).
    smalls_d = nc.declare_dram_parameter("smalls", [128, 40], F32, isOutput=False)
    # cls_b pre-broadcast to [32, 96] on the host: lets the classifier output
    # land as logits [b, t] (32 descriptors x 384B on the store instead of 96
    # x 128B — the store's tail rides the slowest SDMA engine).
    clsb_d = nc.declare_dram_parameter("clsb", [LOCAL_B, T_OUT], F32, isOutput=False)
    out_d = nc.declare_dram_parameter("out", [LOCAL_B, T_OUT], F32, isOutput=True)

    with ExitStack() as ctx:
        member_t = ctx.enter_context(
            nc.sbuf_tensor([128, T_g * G * LOCAL_B], sdt)
        )
        stile = [
            [
                ctx.enter_context(nc.sbuf_tensor(f"stile{si}_{t}", [128, W], sdt))
                for t in range(T_g)
            ]
            for si in range(n_streams)
        ]
        smalls_t = ctx.enter_context(nc.sbuf_tensor([128, 40], F32))
        ident2_t = ctx.enter_context(nc.sbuf_tensor([32, 32], hdt))
        dwT_t = ctx.enter_context(nc.sbuf_tensor([128, 6 * H], hdt))
        cwT_t = ctx.enter_context(nc.sbuf_tensor([128, 6 * T_OUT], hdt))
        pooled_sb = ctx.enter_context(nc.sbuf_tensor([LOCAL_B, H], hdt))
        pooledT_sb = ctx.enter_context(nc.sbuf_tensor([128, 6 * LOCAL_B], hdt))
        hT_sb = ctx.enter_context(nc.sbuf_tensor([128, 6 * LOCAL_B], hdt))
        clsb_t = ctx.enter_context(nc.sbuf_tensor([LOCAL_B, T_OUT], F32))
        logits_sb = ctx.enter_context(nc.sbuf_tensor([LOCAL_B, T_OUT], F32))
        warm_sb = ctx.enter_context(nc.sbuf_tensor([128, 512], sdt))

        pooled_a = ctx.enter_context(nc.psum_tensor([LOCAL_B, 512], F32))
        pooled_b = ctx.enter_context(nc.psum_tensor([LOCAL_B, 512], F32))
        tp = [
            ctx.enter_context(nc.psum_tensor(f"tp{i}", [128, 512], hdt))
            for i in range(3)
        ]
        hps = [
            ctx.enter_context(nc.psum_tensor(f"hps{i}", [128, 512], F32))
            for i in range(2)
        ]
        lps = ctx.enter_context(nc.psum_tensor([LOCAL_B, 512], F32))

        invl_ap = smalls_t[:LOCAL_B, 0:1]
        db6_ap = smalls_t[:, 2:8]

        # Single-stream modes: each stream tile arrives as two half-DMAs with
        # their own sems. The matmuls for the first half run while the second
        # half transfers — and when a slow SDMA engine dribbles the ring
        # tail, only the last half-tile's 8 matmuls wait on it.
        halved = n_streams == 1
        s_member = nc.alloc_semaphore("s_member")
        s_stream = [nc.alloc_semaphore(f"s_stream{t}") for t in range(T_g)]
        s_streamB = [nc.alloc_semaphore(f"s_streamB{t}") for t in range(T_g)]
        s_smalls = nc.alloc_semaphore("s_smalls")
        s_hw = nc.alloc_semaphore("s_hw")  # dwT+cwT (adjacent on one ring:
        # a full-count wait of 32 implies both transfers complete)
        s_pool = nc.alloc_semaphore("s_pool")
        s_scaled = nc.alloc_semaphore("s_scaled")
        s_tr = nc.alloc_semaphore("s_tr")
        s_ptcopy = nc.alloc_semaphore("s_ptcopy")
        s_head = nc.alloc_semaphore("s_head")
        s_tanh = nc.alloc_semaphore("s_tanh")
        s_cls = nc.alloc_semaphore("s_cls")
        s_log = nc.alloc_semaphore("s_log")
        s_out = nc.alloc_semaphore("s_out")
        s_warm = nc.alloc_semaphore("s_warm")

        with nc.Block() as block:

            @block.gpsimd
            def _(gpsimd):
                nc.gpsimd.memset(warm_sb[:], 0.0).then_inc(s_warm, 1)
                # The last group holds only the load-balance remainder: under
                # the p-major packing its real rows occupy partitions
                # [0, p_last). Those above are never transferred (the DMA
                # below skips them) — zero once so the matmuls read 0s
                # (membership is 0 there, but fp16 garbage could be NaN).
                if p_last < 128:
                    for si in range(n_streams):
                        nc.gpsimd.memset(
                            stile[si][T_g - 1][p_last:, :], 0.0
                        ).then_inc(s_warm, 1)

            @block.sync
            def _(sync):
                # FIFO ring: group-0 slice of member first, then stream tile
                # 0, then the rest in consumption order — the first matmul
                # needs only the first two transfers (~1.6MB).
                g0 = G * LOCAL_B
                sync.dma_start(
                    out=member_t[:, :g0], in_=member_d[:, :g0]
                ).then_inc(s_member, 16)
                def stream_tile(t):
                    pl = p_last if t == T_g - 1 else 128
                    if halved:
                        hw2 = W // 2
                        sync.dma_start(
                            out=stile[0][t][:pl, :hw2],
                            in_=streams_d[0][t][:pl, :hw2],
                        ).then_inc(s_stream[t], 16)
                        sync.dma_start(
                            out=stile[0][t][:pl, hw2:],
                            in_=streams_d[0][t][:pl, hw2:],
                        ).then_inc(s_streamB[t], 16)
                    else:
                        for si in range(n_streams):
                            sync.dma_start(
                                out=stile[si][t][:pl, :], in_=streams_d[si][t][:pl, :]
                            ).then_inc(s_stream[t], 16)

                stream_tile(0)
                sync.dma_start(
                    out=member_t[:, g0:], in_=member_d[:, g0:]
                ).then_inc(s_member1, 16)
                for t in range(1, T_g):
                    stream_tile(t)
                    if t == 2:
                        # tiny consts ride mid-ring: early enough that their
                        # last descriptors never trail on the slowest SDMA
                        # engine (which stalls the head), late enough that
                        # their issue slots don't delay the ramp-phase
                        # transfers the first matmuls wait on.
                        sync.dma_start(
                            out=smalls_t[:], in_=smalls_d[:]
                        ).then_inc(s_smalls, 16)
                        sync.dma_start(
                            out=ident2_t[:], in_=ident2_d[:]
                        ).then_inc(s_smalls, 16)
                        sync.dma_start(out=clsb_t[:], in_=clsb_d[:]).then_inc(
                            s_smalls, 16
                        )
                sync.dma_start(out=dwT_t[:], in_=dwT_d[:]).then_inc(s_hw, 16)
                sync.dma_start(out=cwT_t[:], in_=cwT_d[:]).then_inc(s_hw, 16)
                # output store (waits for the DVE bias-add)
                sync.wait_ge(s_log, 1)
                sync.dma_start(out=out_d[:], in_=logits_sb[:]).then_inc(s_out, 16)
                sync.wait_ge(s_out, 16)

            @block.tensor
            def _(tensor):
                # HAM management: the PE clock-gate re-throttles to 4/8 after
                # an idle activity window (~3.4us). The kernel is DMA-paced,
                # so the PE would idle ~1us per group — enough, with bad
                # window phase, to oscillate between 1.2 and 2.4 GHz. Filler
                # matmuls on a zeroed scratch tile (into the hps bank, which
                # the dense phase later resets with start=True) keep the PE
                # busy through every wait.
                def filler(n):
                    for _ in range(n):
                        nc.tensor.matmul(
                            hps[0][:, :512],
                            warm_sb[:, :128],
                            warm_sb[:, :512],
                            start=True, stop=True,
                        )

                tensor.wait_ge(s_warm, 1 + (n_streams if p_last < 128 else 0))
                filler(12)
                n_mm = T_g * G * n_streams
                i_mm = 0
                last_mm = None
                for t in range(T_g):
                    if t == 0:
                        tensor.wait_ge(s_member, 16)
                    else:
                        filler(4)
                        tensor.wait_ge(s_member1, 16)
                    for q in range(G):
                        if q == 0:
                            tensor.wait_ge(s_stream[t], 16 * n_streams)
                        elif halved and q == G // 2:
                            tensor.wait_ge(s_streamB[t], 16)
                        k = t * G + q
                        lhsT = member_t[:, k * LOCAL_B : (k + 1) * LOCAL_B]
                        for si in range(n_streams):
                            st = stile[si][t]
                            first, last = i_mm == 0, i_mm == n_mm - 1
                            nc.tensor.matmul(
                                pooled_a[:, :512], lhsT,
                                st[:, q * H : q * H + 512],
                                start=first, stop=last,
                            )
                            last_mm = nc.tensor.matmul(
                                pooled_b[:, : H - 512], lhsT,
                                st[:, q * H + 512 : (q + 1) * H],
                                start=first, stop=last,
                            )
                            i_mm += 1
                last_mm.then_inc(s_pool, 1)
                # transposes (need DVE scale + the identity matrix)
                filler(3)
                tensor.wait_ge(s_smalls, 48)
                for c in range(6):
                    tensor.wait_ge(s_scaled, 1 if c < 4 else 2)
                    if c >= 3:
                        tensor.wait_ge(s_ptcopy, c - 2)
                    nc.tensor.transpose(
                        tp[c % 3][:, :LOCAL_B],
                        pooled_sb[:, c * 128 : (c + 1) * 128],
                        ident2_t[:],
                    ).then_inc(s_tr, 1)
                # dense layer
                filler(2)
                tensor.wait_ge(s_ptcopy, 6)
                tensor.wait_ge(s_hw, 32)
                for jg in range(6):
                    if jg >= 2:
                        tensor.wait_ge(s_tanh, jg - 1)
                    for c in range(6):
                        mm = nc.tensor.matmul(
                            hps[jg % 2][:, :LOCAL_B],
                            dwT_t[:, c * H + jg * 128 : c * H + (jg + 1) * 128],
                            pooledT_sb[:, c * LOCAL_B : (c + 1) * LOCAL_B],
                            start=(c == 0), stop=(c == 5),
                        )
                    mm.then_inc(s_head, 1)
                # classifier
                # classifier: logits[b, t] — hT chunk is the stationary
                # operand so the output lands batch-major.
                tensor.wait_ge(s_cwT, 16)
                for jg in range(6):
                    tensor.wait_ge(s_tanh, jg + 1)
                    mm = nc.tensor.matmul(
                        lps[:, :T_OUT],
                        hT_sb[:, jg * LOCAL_B : (jg + 1) * LOCAL_B],
                        cwT_t[:, jg * T_OUT : (jg + 1) * T_OUT],
                        start=(jg == 0), stop=(jg == 5),
                    )
                mm.then_inc(s_cls, 1)

            @block.vector
            def _(vector):
                vector.wait_ge(s_smalls, 48)
                vector.wait_ge(s_pool, 1)
                nc.vector.tensor_scalar_mul(
                    pooled_sb[:, 0:512], pooled_a[:, :512], invl_ap
                ).then_inc(s_scaled, 1)
                nc.vector.tensor_scalar_mul(
                    pooled_sb[:, 512:H], pooled_b[:, : H - 512], invl_ap
                ).then_inc(s_scaled, 1)
                for c in range(6):
                    vector.wait_ge(s_tr, c + 1)
                    nc.vector.tensor_copy(
                        pooledT_sb[:, c * LOCAL_B : (c + 1) * LOCAL_B],
                        tp[c % 3][:, :LOCAL_B],
                    ).then_inc(s_ptcopy, 1)
                vector.wait_ge(s_cls, 1)
                nc.vector.tensor_add(
                    logits_sb[:], lps[:, :T_OUT], clsb_t[:]
                ).then_inc(s_log, 1)

            @block.scalar
            def _(scalar):
                scalar.wait_ge(s_smalls, 48)
                for jg in range(6):
                    scalar.wait_ge(s_head, jg + 1)
                    nc.scalar.activation(
                        hT_sb[:, jg * LOCAL_B : (jg + 1) * LOCAL_B],
                        hps[jg % 2][:, :LOCAL_B],
                        mybir.ActivationFunctionType.Tanh,
                        bias=db6_ap[:, jg : jg + 1],
                    ).then_inc(s_tanh, 1)

    nc.compile()
    return nc


def kernel(hidden_states, pivot_len_list, dense_w, dense_b, cls_w, cls_b):
    global last_results
    hs = np.ascontiguousarray(np.asarray(hidden_states, dtype=np.float32))
    lens = np.asarray(pivot_len_list).astype(np.int64)
    dense_w = np.asarray(dense_w, dtype=np.float32)
    dense_b = np.asarray(dense_b, dtype=np.float32)
    cls_w = np.asarray(cls_w, dtype=np.float32)
    cls_b = np.asarray(cls_b, dtype=np.float32)
    assert hs.shape == (B, S, H), hs.shape
    assert lens.shape == (B,), lens.shape

    if MODE == "f8":
        return _kernel_f8(hs, lens, dense_w, dense_b, cls_w, cls_b)

    mode = MODE
    np_sdt = np.float16 if mode == "f16" else ml_dtypes.bfloat16

    # ---- assign samples to cores: greedy LPT with a hard 32-per-core cap
    order = np.argsort(-lens, kind="stable")
    core_samples = [[] for _ in range(N_CORES)]
    load = np.zeros(N_CORES, dtype=np.int64)
    for b in order:
        open_cores = [c for c in range(N_CORES) if len(core_samples[c]) < LOCAL_B]
        c = min(open_cores, key=lambda c: load[c])
        core_samples[c].append(int(b))
        load[c] += int(lens[b])
    T_g = max(1, -(-int(load.max()) // ROWS_PER_GROUP))

    impl = IMPL
    # Partitions actually occupied in the final (remainder) group under the
    # p-major packing; the rest of that tile is padding and never transferred.
    rows_last = int(load.max()) - (T_g - 1) * ROWS_PER_GROUP
    # partition offsets must be 32-aligned (engine base-partition constraint)
    p_last = min(128, max(32, 32 * -(--(-rows_last // G) // 32)))
    key = (T_g, mode, impl, p_last)
    if key not in _cache:
        if impl == "raw":
            _cache[key] = _build_program_raw(T_g, mode, p_last)
        else:
            _cache[key] = _build_program(T_g, mode)
    nc = _cache[key]

    # ---- shared (replicated) head tensors
    dwT_host = np.empty((128, 6 * H), np.float32)
    for c in range(6):
        dwT_host[:, c * H : (c + 1) * H] = dense_w[:, c * 128 : (c + 1) * 128].T
    cwT_host = np.empty((128, 6 * T_OUT), np.float32)
    for jg in range(6):
        cwT_host[:, jg * T_OUT : (jg + 1) * T_OUT] = cls_w[:, jg * 128 : (jg + 1) * 128].T
    db6_host = np.ascontiguousarray(dense_b.reshape(6, 128).T)
    cb1_host = np.ascontiguousarray(cls_b.reshape(T_OUT, 1))
    ident_host = np.eye(32, dtype=np.float32)

    # ---- per-core packing
    hs2 = hs.reshape(B * S, H)
    NR = T_g * ROWS_PER_GROUP
    in_maps = []
    for c in range(N_CORES):
        samples = core_samples[c]
        lens_c = lens[samples]
        idx = np.concatenate(
            [np.arange(b * S + 1, b * S + 1 + lens[b]) for b in samples]
        )
        n = idx.size
        packed = np.zeros((NR, H), np.float32)
        packed[:n] = hs2[idx]
        if mode == "f32x2":
            hi = packed.astype(ml_dtypes.bfloat16)
            lo = (packed - hi.astype(np.float32)).astype(ml_dtypes.bfloat16)
            stream_arrays = [hi, lo]
        else:
            stream_arrays = [packed.astype(np_sdt)]

        j = np.arange(n)
        tt = j // ROWS_PER_GROUP
        p = (j % ROWS_PER_GROUP) // G
        q = j % G
        local_b = np.repeat(np.arange(LOCAL_B), lens_c)
        mem = np.zeros((128, T_g * G * LOCAL_B), np_sdt)
        mem[p, (tt * G + q) * LOCAL_B + local_b] = np_sdt(1.0)

        invl_host = (1.0 / lens_c.astype(np.float32)).reshape(LOCAL_B, 1)
        im = {"member": mem, "dwT": dwT_host, "cwT": cwT_host}
        if impl == "raw":
            np_hdt = np.float32 if mode == "f32x2" else np_sdt
            im["dwT"] = dwT_host.astype(np_hdt)
            im["cwT"] = cwT_host.astype(np_hdt)
            im["ident2"] = np.eye(32, dtype=np.float32).astype(np_hdt)
            im["clsb"] = np.ascontiguousarray(
                np.broadcast_to(cls_b, (LOCAL_B, T_OUT)).astype(np.float32)
            )
            smalls = np.zeros((128, 40), np.float32)
            smalls[:LOCAL_B, 0:1] = invl_host
            smalls[:T_OUT, 1:2] = cb1_host
            smalls[:, 2:8] = db6_host
            smalls[:32, 8:40] = ident_host
            im["smalls"] = smalls
        else:
            im["db6"] = db6_host
            im["cb1"] = cb1_host
            im["invl"] = invl_host
            im["ident"] = ident_host
        for i, arr in enumerate(stream_arrays):
            im[f"hs{i}"] = arr.reshape(T_g, 128, G * H)
        in_maps.append(im)

    trace = bool(os.environ.get("KERNEL_TRACE"))
    try:
        res = bass_utils.run_bass_kernel_spmd(
            nc, in_maps, list(range(N_CORES)), trace=trace
        )
    except Exception:
        # Transient NRT device errors (e.g. NRT_EXEC_UNIT_UNRECOVERABLE after
        # many back-to-back launches) clear on retry.
        res = bass_utils.run_bass_kernel_spmd(
            nc, in_maps, list(range(N_CORES)), trace=trace
        )
    last_results = res

    logits = np.zeros((B, T_OUT), np.float32)
    for c in range(N_CORES):
        o = res.results[c]["out"]
        logits[core_samples[c], :] = o if impl == "raw" else o.T
    return logits



# revision 33
# speedup vs baseline: 1.0890x; 1.0890x over previous
"""Trainium2 Bass kernel: ragged mean-pool over [1, len_i] + Linear->tanh->Linear head.

Strategy (pure data parallel over batch, 8 NeuronCores):
  * Host: balance the 256 samples across 8 cores (32 each) by total row count,
    gather only the needed rows hidden_states[b, 1:len_b+1, :] into a packed
    dense array per core (the ragged/masked structure becomes a small 0/1
    "membership" matrix), and encode rows compactly (fp16 by default).
  * Device: stream packed row-tiles [128, 8*768]; for each 128-row subtile do
    pooled[b,h] += member[r,b] * rows[r,h] as a PE matmul with the membership
    matrix as the stationary operand, accumulating all tiles into one PSUM
    region. Then scale by 1/len, transpose, and run the tiny 768x768 tanh head
    and 96x768 classifier fully on-chip. Output is logits^T [96, 32] per core.
  * Host: scatter per-core outputs back to the full [256, 96] logits.

The compiled program depends only on (T_g, mode) where T_g = number of
1024-row groups per core -- not on the actual lengths -- so recompiles are
rare. All raggedness lives in data (packing + membership).
"""

import os
from contextlib import ExitStack

import numpy as np
import ml_dtypes

import concourse.bass as bass
import concourse.mybir as mybir
from concourse import bacc, bass_utils, tile

B, S, H, T_OUT = 256, 512, 768, 96
N_CORES = 8
LOCAL_B = B // N_CORES        # 32 samples per core
G = 8                         # packed rows per partition per group tile
ROWS_PER_GROUP = 128 * G      # 1024
F32 = mybir.dt.float32

# Row encodings: "f8" (1B/elem e4m3 + per-sample error-feedback + one mop-up
# residual row, ~6e-4 rel err, DoubleRow double-pumped PE), "f16" (2B/elem,
# ~2e-4 rel err), "f32x2" (bf16 hi+lo pair, 4B/elem, ~2e-6 rel err), "bf16"
# (2B/elem, ~1.4e-3 rel err).
MODE = os.environ.get("KERNEL_MODE", "f8")
# f8 pooling matmul width: 1 = double-wide DoubleRow (rhs free 1024, 2 matmuls
# per subtile pair), 0 = 3 matmuls of 512. Settable per-call via kernel.WIDE.
WIDE = int(os.environ.get("KERNEL_WIDE", "1"))
# f8 pooled-scale split (DVE [0:256] + ACT [256:512],[512:768]) HANGS the
# device -- suspected ACT+DVE concurrent read of the same PSUM bank. Hard
# disabled; the incumbent keeps the two engines on separate pooled banks.
SC3 = 0
# f8 stream tail chunking: 1 = taper the last chunks ([6,4,2] subtiles), 0 =
# uniform 8-subtile chunks. Settable per-call via kernel.TAPER.
TAPER = int(os.environ.get("KERNEL_TAPER", "0"))
# "raw" = hand-synchronized Bacc program (no Tile scheduler, minimal
# semaphore traffic and no kernel-tail sem-reset butterfly); "tile" = the
# TileContext-scheduled variant.
IMPL = os.environ.get("KERNEL_IMPL", "raw")

_cache: dict = {}
last_results = None  # BassKernelResults of the most recent run (for test.py)


def _build_program(T_g: int, mode: str) -> bass.Bass:
    sdt = mybir.dt.float16 if mode == "f16" else mybir.dt.bfloat16
    n_streams = 2 if mode == "f32x2" else 1
    W = G * H  # free-dim width of a group tile

    # Bacc (not raw Bass): its compile() pass splits multi-semaphore waits
    # into EventSemaphore chains — hardware allows at most 1 wait per
    # instruction — and moves matmul waits onto ldweights.
    nc = bacc.Bacc()
    streams = [
        nc.declare_dram_parameter(f"hs{i}", [T_g, 128, W], sdt, isOutput=False)
        for i in range(n_streams)
    ]
    member = nc.declare_dram_parameter(
        "member", [128, T_g * G * LOCAL_B], sdt, isOutput=False
    )
    dwT = nc.declare_dram_parameter("dwT", [128, 6 * H], F32, isOutput=False)
    cwT = nc.declare_dram_parameter("cwT", [128, 6 * T_OUT], F32, isOutput=False)
    db6 = nc.declare_dram_parameter("db6", [128, 6], F32, isOutput=False)
    cb1 = nc.declare_dram_parameter("cb1", [T_OUT, 1], F32, isOutput=False)
    invl = nc.declare_dram_parameter("invl", [LOCAL_B, 1], F32, isOutput=False)
    ident = nc.declare_dram_parameter("ident", [32, 32], F32, isOutput=False)
    out = nc.declare_dram_parameter("out", [T_OUT, LOCAL_B], F32, isOutput=True)

    with ExitStack() as ctx:
        tc = ctx.enter_context(tile.TileContext(nc))
        const_pool = ctx.enter_context(tc.tile_pool(name="const", bufs=1))
        # All group tiles resident at once (single-stream modes fit: T_g * 12KB
        # per partition). Slot reuse would attach 3 semaphore waits to the
        # reload DMAs, which the DMA instruction encoding cannot carry.
        in_bufs = T_g * n_streams if n_streams == 1 else 3
        in_pool = ctx.enter_context(tc.tile_pool(name="inp", bufs=in_bufs))
        sb_pool = ctx.enter_context(tc.tile_pool(name="sb", bufs=1))
        ps_pooled = ctx.enter_context(tc.tile_pool(name="psp", bufs=1, space="PSUM"))
        ps_small = ctx.enter_context(tc.tile_pool(name="pss", bufs=2, space="PSUM"))

        # DMA order matters: queues drain in emission order, and the pooling
        # matmuls only need `member` + their stream tile. Load those first;
        # the head weights (dwT/cwT, ~2.7MB) are consumed only after all
        # pooling, so they stream in behind and overlap the pooling phase.
        member_t = const_pool.tile([128, T_g * G * LOCAL_B], sdt)
        nc.sync.dma_start(member_t[:], member[:])

        all_stiles = []
        for t in range(T_g):
            stiles = []
            for si, s in enumerate(streams):
                st = in_pool.tile([128, W], sdt, tag=f"s{si}")
                nc.sync.dma_start(st[:], s[t])
                stiles.append(st)
            all_stiles.append(stiles)

        invl_t = const_pool.tile([LOCAL_B, 1], F32)
        nc.sync.dma_start(invl_t[:], invl[:])
        ident_t = const_pool.tile([32, 32], F32)
        nc.sync.dma_start(ident_t[:], ident[:])
        dwT_t = const_pool.tile([128, 6 * H], F32)
        nc.sync.dma_start(dwT_t[:], dwT[:])
        cwT_t = const_pool.tile([128, 6 * T_OUT], F32)
        nc.sync.dma_start(cwT_t[:], cwT[:])
        db6_t = const_pool.tile([128, 6], F32)
        nc.sync.dma_start(db6_t[:], db6[:])
        cb1_t = const_pool.tile([T_OUT, 1], F32)
        nc.sync.dma_start(cb1_t[:], cb1[:])

        # Pre-touch small const tiles on the engine that will consume them:
        # several ISA instruction encodings carry only ONE semaphore wait, so
        # the consuming op must not need both its data-producer wait and a
        # const-DMA wait. Touching the const here advances that engine's
        # observed clock past the const DMA, and the later wait is elided.
        scratch = const_pool.tile([128, 8], F32)
        nc.vector.tensor_copy(scratch[:LOCAL_B, 0:1], invl_t[:])
        nc.vector.tensor_copy(scratch[:T_OUT, 1:2], cb1_t[:])
        nc.scalar.activation(
            scratch[:, 2:8], db6_t[:], mybir.ActivationFunctionType.Copy
        )

        # ---- ragged pooling: pooled[b, h] = sum over packed rows r of
        #      member[r, b] * row[r, h], accumulated in PSUM over all tiles.
        pooled_a = ps_pooled.tile([LOCAL_B, 512], F32, tag="pa")
        pooled_b = ps_pooled.tile([LOCAL_B, H - 512], F32, tag="pb")
        n_mm = T_g * G * n_streams  # matmuls per PSUM region
        i_mm = 0
        for t in range(T_g):
            stiles = all_stiles[t]
            for q in range(G):
                k = t * G + q
                lhsT = member_t[:, k * LOCAL_B : (k + 1) * LOCAL_B]
                for st in stiles:
                    first, last = i_mm == 0, i_mm == n_mm - 1
                    nc.tensor.matmul(
                        pooled_a[:], lhsT, st[:, q * H : q * H + 512],
                        start=first, stop=last,
                    )
                    nc.tensor.matmul(
                        pooled_b[:], lhsT, st[:, q * H + 512 : (q + 1) * H],
                        start=first, stop=last,
                    )
                    i_mm += 1

        # ---- mean: scale each sample's partition by 1/len
        pooled_sb = sb_pool.tile([LOCAL_B, H], F32)
        nc.vector.tensor_scalar_mul(pooled_sb[:, 0:512], pooled_a[:], invl_t[:])
        nc.vector.tensor_scalar_mul(pooled_sb[:, 512:H], pooled_b[:], invl_t[:])

        # ---- transpose pooled [32, 768] -> pooledT [768, 32] via PE
        pooledT_sb = sb_pool.tile([128, 6 * LOCAL_B], F32)
        for c in range(6):
            tp = ps_small.tile([128, LOCAL_B], F32, tag="tp")
            nc.tensor.transpose(
                tp[:], pooled_sb[:, c * 128 : (c + 1) * 128], ident_t[:]
            )
            nc.vector.tensor_copy(pooledT_sb[:, c * LOCAL_B : (c + 1) * LOCAL_B], tp[:])

        # ---- dense layer + tanh: hT[j, b] = tanh(dense_b[j] + sum_h dwT[h, j] pooledT[h, b])
        hT_sb = sb_pool.tile([128, 6 * LOCAL_B], F32)
        for jg in range(6):
            hps = ps_small.tile([128, LOCAL_B], F32, tag="hps")
            for c in range(6):
                nc.tensor.matmul(
                    hps[:],
                    dwT_t[:, c * H + jg * 128 : c * H + (jg + 1) * 128],
                    pooledT_sb[:, c * LOCAL_B : (c + 1) * LOCAL_B],
                    start=(c == 0), stop=(c == 5),
                )
            nc.scalar.activation(
                hT_sb[:, jg * LOCAL_B : (jg + 1) * LOCAL_B],
                hps[:],
                mybir.ActivationFunctionType.Tanh,
                bias=db6_t[:, jg : jg + 1],
            )

        # ---- classifier: logitsT[t, b] = cls_b[t] + sum_j cwT[j, t] hT[j, b]
        lps = ps_small.tile([T_OUT, LOCAL_B], F32, tag="lps")
        for jg in range(6):
            nc.tensor.matmul(
                lps[:],
                cwT_t[:, jg * T_OUT : (jg + 1) * T_OUT],
                hT_sb[:, jg * LOCAL_B : (jg + 1) * LOCAL_B],
                start=(jg == 0), stop=(jg == 5),
            )
        logits_sb = sb_pool.tile([T_OUT, LOCAL_B], F32)
        nc.vector.tensor_scalar_add(logits_sb[:], lps[:], cb1_t[:])
        # SWDGE store: lands on a fresh DMASW sem lane, so it carries only the
        # DVE wait (every encoding has a single wait slot).
        nc.gpsimd.dma_start(out[:], logits_sb[:])

    nc.compile()
    return nc


F8 = mybir.dt.float8e4
P8 = 128                      # stream partitions in f8 mode (124 was tried to
                              # starve slow SDMA engine 15, but non-128
                              # partition DMAs fall off the fast descriptor
                              # path: 2x slower overall)
NP_F8 = ml_dtypes.float8_e4m3  # IEEE-style e4m3 (max 240) == TRN FP8_EXP4
F16 = mybir.dt.float16


def _build_program_f8(
    n_full: int, g_last: int, wide: bool = False, taper: bool = False,
    sc3: bool = False,
) -> bass.Bass:
    """fp8 variant: e4m3 streams + membership, DoubleRow double-pumped pooling
    matmuls (256-row contraction per instruction), fp16 head.

    Layout: one flat stream tensor sdata [128, K_cols, H] (K_cols 128-row
    subtiles; groups of 8 subtiles share a membership accumulation pattern,
    the optional last group holds g_last even subtiles). All padding rows are
    host-zeroed (fp8 0x00) so there are no device memsets and no gpsimd use.

    DMA: the stream rides the Sync HWDGE ring in a few LARGE chunks (small
    first chunk for a fast pipeline start) -- per-instruction descriptor
    generation (~0.7-1.3us) otherwise cannot keep 16 SDMA engines fed at
    ~400GB/s with halved (fp8) per-instruction bytes. Everything else
    (member, smalls, head weights) rides the Scalar/ACT HWDGE ring
    concurrently, so head weights arrive early without delaying the stream
    tail."""
    n_groups = n_full + (1 if g_last else 0)
    g_of = lambda t: 8 if t < n_full else g_last
    K_cols = 8 * n_full + g_last  # total 128-row subtiles
    DR = mybir.MatmulPerfMode.DoubleRow

    # stream chunk boundaries in subtile units: one chunk per group. Fine
    # granularity keeps the tail exposure small (only the last small chunk
    # gates the final pooling) while ~0.8us/instruction descriptor-gen still
    # stays well ahead of the ~1.9us/group transfer time.
    if taper:
        TAPS = {12: [6, 4, 2], 10: [4, 4, 2], 8: [4, 2, 2], 6: [4, 2],
                4: [2, 2], 2: [2]}
        bnds = [0]
        while K_cols - bnds[-1] > 12:
            bnds.append(bnds[-1] + 8)
        for step in TAPS.get(K_cols - bnds[-1], [K_cols - bnds[-1]]):
            bnds.append(bnds[-1] + step)
    else:
        bnds = [0]
        while bnds[-1] < K_cols:
            bnds.append(min(bnds[-1] + 8, K_cols))
    n_chunks = len(bnds) - 1
    chunk_of_sub = {}
    for i in range(n_chunks):
        for s in range(bnds[i], bnds[i + 1]):
            chunk_of_sub[s] = i

    nc = bacc.Bacc(enable_partition_id=False)
    hs_d = nc.declare_dram_parameter("hs", [P8, K_cols * H], F8, isOutput=False)
    member_d = nc.declare_dram_parameter(
        "member", [P8, K_cols * LOCAL_B], F8, isOutput=False
    )
    dwT_d = nc.declare_dram_parameter("dwT", [128, 6 * H], F16, isOutput=False)
    cwT_d = nc.declare_dram_parameter("cwT", [128, 6 * T_OUT], F16, isOutput=False)
    ident2_d = nc.declare_dram_parameter("ident2", [32, 32], F16, isOutput=False)
    # smalls blob [128, 8] f32: col 0 invl (rows 0-31), cols 2-7 dense_b chunks
    smalls_d = nc.declare_dram_parameter("smalls", [128, 8], F32, isOutput=False)
    clsb_d = nc.declare_dram_parameter("clsb", [LOCAL_B, T_OUT], F32, isOutput=False)
    out_d = nc.declare_dram_parameter("out", [LOCAL_B, T_OUT], F32, isOutput=True)

    with ExitStack() as ctx:
        member_t = ctx.enter_context(
            nc.sbuf_tensor([P8, K_cols, LOCAL_B], F8)
        )
        sdata = ctx.enter_context(nc.sbuf_tensor([P8, K_cols, H], F8))
        smalls_t = ctx.enter_context(nc.sbuf_tensor([128, 8], F32))
        ident2_t = ctx.enter_context(nc.sbuf_tensor([32, 32], F16))
        dwT_t = ctx.enter_context(nc.sbuf_tensor([128, 6 * H], F16))
        cwT_t = ctx.enter_context(nc.sbuf_tensor([128, 6 * T_OUT], F16))
        pooled_sb = ctx.enter_context(nc.sbuf_tensor([LOCAL_B, H], F16))
        pooledT_sb = ctx.enter_context(nc.sbuf_tensor([128, 6 * LOCAL_B], F16))
        hT_sb = ctx.enter_context(nc.sbuf_tensor([128, 6 * LOCAL_B], F16))
        clsb_t = ctx.enter_context(nc.sbuf_tensor([LOCAL_B, T_OUT], F32))
        logits_sb = ctx.enter_context(nc.sbuf_tensor([LOCAL_B, T_OUT], F32))

        pooled_a = ctx.enter_context(nc.psum_tensor([LOCAL_B, 512], F32))
        pooled_b = ctx.enter_context(nc.psum_tensor([LOCAL_B, 512], F32))
        tp = [
            ctx.enter_context(nc.psum_tensor(f"tp{i}", [128, 512], F16))
            for i in range(3)
        ]
        hps = [
            ctx.enter_context(nc.psum_tensor(f"hps{i}", [128, 512], F32))
            for i in range(2)
        ]
        lps = ctx.enter_context(nc.psum_tensor([LOCAL_B, 512], F32))

        invl_ap = smalls_t[:LOCAL_B, 0:1]
        db6_ap = smalls_t[:, 2:8]

        s_member = nc.alloc_semaphore("s_member")
        s_chunk = [nc.alloc_semaphore(f"s_chunk{i}") for i in range(n_chunks)]
        s_smalls = nc.alloc_semaphore("s_smalls")
        s_hw = nc.alloc_semaphore("s_hw")  # dwT+cwT (adjacent on one ring:
        # a full-count wait of 32 implies both transfers complete)
        s_pool = nc.alloc_semaphore("s_pool")
        s_scA = nc.alloc_semaphore("s_scA")
        s_scB = nc.alloc_semaphore("s_scB")
        s_tr = nc.alloc_semaphore("s_tr")
        s_ptcopy = nc.alloc_semaphore("s_ptcopy")
        s_head = nc.alloc_semaphore("s_head")
        s_tanh = nc.alloc_semaphore("s_tanh")
        s_cls = nc.alloc_semaphore("s_cls")
        s_log = nc.alloc_semaphore("s_log")
        s_out = nc.alloc_semaphore("s_out")

        with nc.Block() as block:

            @block.sync
            def _(sync):
                for i in range(n_chunks):
                    a, b = bnds[i], bnds[i + 1]
                    sync.dma_start(
                        out=sdata[:, a:b, :], in_=hs_d[:, a * H : b * H]
                    ).then_inc(s_chunk[i], 16)
                # Head weights ride the sync queue AFTER every stream chunk:
                # the queue drains FIFO per engine, so they never steal
                # bandwidth mid-stream (a mid-stream transfer delays a chunk
                # by ~4.5us, idles the PE past the HAM window, and the
                # re-throttled PE then runs the rest of the pooling at half
                # clock). They land ~3us after the last chunk, just before
                # the dense layer needs them.
                sync.dma_start(out=dwT_t[:], in_=dwT_d[:]).then_inc(s_hw, 16)
                sync.dma_start(out=cwT_t[:], in_=cwT_d[:]).then_inc(s_hw, 16)
                sync.wait_ge(s_log, 1)
                sync.dma_start(out=out_d[:], in_=logits_sb[:]).then_inc(s_out, 16)
                sync.wait_ge(s_out, 16)

            @block.scalar
            def _(scalar):
                # second HWDGE ring: member + small consts + head weights
                scalar.dma_start(out=member_t[:], in_=member_d[:]).then_inc(
                    s_member, 16
                )
                scalar.dma_start(out=smalls_t[:], in_=smalls_d[:]).then_inc(
                    s_smalls, 16
                )
                scalar.dma_start(out=ident2_t[:], in_=ident2_d[:]).then_inc(
                    s_smalls, 16
                )
                scalar.dma_start(out=clsb_t[:], in_=clsb_d[:]).then_inc(s_smalls, 16)
                scalar.wait_ge(s_smalls, 48)
                # dummy tanh: pulls the ~1.3us ACT_TABLE_LOAD off the critical
                # path. Overwritten by the real jg=0 tanh later.
                nc.scalar.activation(
                    hT_sb[:, 0:1], smalls_t[:, 0:1],
                    mybir.ActivationFunctionType.Tanh,
                ).then_inc(s_tanh, 1)
                # pooled scale, ACT side (concurrent with the DVE side)
                scalar.wait_ge(s_pool, 1)
                if sc3:
                    nc.scalar.activation(
                        pooled_sb[:, 256:512], pooled_a[:, 256:512],
                        mybir.ActivationFunctionType.Copy,
                        scale=invl_ap,
                    ).then_inc(s_scB, 1)
                nc.scalar.activation(
                    pooled_sb[:, 512:H], pooled_b[:, : H - 512],
                    mybir.ActivationFunctionType.Copy,
                    scale=invl_ap,
                ).then_inc(s_scB, 1)
                for jg in range(6):
                    scalar.wait_ge(s_head, jg + 1)
                    nc.scalar.activation(
                        hT_sb[:, jg * LOCAL_B : (jg + 1) * LOCAL_B],
                        hps[jg % 2][:, :LOCAL_B],
                        mybir.ActivationFunctionType.Tanh,
                        bias=db6_ap[:, jg : jg + 1],
                    ).then_inc(s_tanh, 1)

            @block.tensor
            def _(tensor):
                # HAM keep-warm fillers: matmuls on (possibly uninitialized)
                # stream bytes into the hps scratch bank; any NaNs land in
                # PSUM that the dense phase later resets with start=True.
                def filler(n):
                    for _ in range(n):
                        nc.tensor.matmul(
                            hps[0][:, :512],
                            sdata[:, 0:1, 0:128],
                            sdata[:, 0:1, 0:512],
                            start=True, stop=True,
                        )

                filler(12)
                tensor.wait_ge(s_member, 16)
                n_pairs = K_cols // 2
                i_mm = 0
                last_mm = None
                cur_chunk = -1
                for t in range(n_groups):
                    g = g_of(t)
                    k_off = 8 * t
                    for qp in range(0, g, 2):
                        need = chunk_of_sub[k_off + qp + 1]
                        if need > cur_chunk:
                            filler(2)
                            tensor.wait_ge(s_chunk[need], 16)
                            cur_chunk = need
                        first, last = i_mm == 0, i_mm == n_pairs - 1
                        if first:
                            # start=True zeroes PSUM at region (bank)
                            # granularity: open each bank with exactly ONE
                            # full-width start=True write (plain fp8 matmuls
                            # over the first subtile pair); everything after
                            # accumulates with start=False.
                            for sub in (0, 1):
                                nc.tensor.matmul(
                                    pooled_a[:, 0:512],
                                    member_t[:, k_off + sub : k_off + sub + 1, :],
                                    sdata[:, k_off + sub : k_off + sub + 1, 0:512],
                                    start=(sub == 0), stop=False,
                                )
                                last_mm = nc.tensor.matmul(
                                    pooled_b[:, 0:256],
                                    member_t[:, k_off + sub : k_off + sub + 1, :],
                                    sdata[:, k_off + sub : k_off + sub + 1, 512:768],
                                    start=(sub == 0), stop=last,
                                )
                        else:
                            lhsT = member_t[:, k_off + qp : k_off + qp + 2, :]
                            pair = sdata[:, k_off + qp : k_off + qp + 2, :]
                            if wide:
                                # rhs free 1024 (fp8 moving max per HW, above
                                # bass's unused fp32-era 512 constant): 2
                                # matmuls per pair, bank A one full region
                                nc.tensor.matmul(
                                    pooled_a[:, 0:512], lhsT, pair[:, :, 0:512],
                                    start=False, stop=last, perf_mode=DR,
                                )
                                last_mm = nc.tensor.matmul(
                                    pooled_b[:, 0:256], lhsT, pair[:, :, 512:768],
                                    start=False, stop=last, perf_mode=DR,
                                )
                            else:
                                for out_ap, h0 in (
                                    (pooled_a[:, 0:256], 0),
                                    (pooled_a[:, 256:512], 256),
                                    (pooled_b[:, 0:256], 512),
                                ):
                                    last_mm = nc.tensor.matmul(
                                        out_ap, lhsT,
                                        pair[:, :, h0 : h0 + 256],
                                        start=False, stop=last, perf_mode=DR,
                                    )
                        i_mm += 1
                last_mm.then_inc(s_pool, 1)
                # transposes (need the scales + the identity matrix)
                filler(2)
                tensor.wait_ge(s_smalls, 48)
                for c in range(6):
                    if sc3:
                        if c < 2:
                            tensor.wait_ge(s_scA, 1)
                        else:
                            tensor.wait_ge(s_scB, 1 if c < 4 else 2)
                    else:
                        tensor.wait_ge(s_scA if c < 4 else s_scB, 1)
                    if c >= 3:
                        tensor.wait_ge(s_ptcopy, c - 2)
                    nc.tensor.transpose(
                        tp[c % 3][:, :LOCAL_B],
                        pooled_sb[:, c * 128 : (c + 1) * 128],
                        ident2_t[:],
                    ).then_inc(s_tr, 1)
                # dense layer
                filler(2)
                tensor.wait_ge(s_ptcopy, 6)
                tensor.wait_ge(s_hw, 32)
                for jg in range(6):
                    if jg >= 2:
                        tensor.wait_ge(s_tanh, jg)
                    for c in range(6):
                        mm = nc.tensor.matmul(
                            hps[jg % 2][:, :LOCAL_B],
                            dwT_t[:, c * H + jg * 128 : c * H + (jg + 1) * 128],
                            pooledT_sb[:, c * LOCAL_B : (c + 1) * LOCAL_B],
                            start=(c == 0), stop=(c == 5),
                        )
                    mm.then_inc(s_head, 1)
                # classifier: logits[b, t] -- hT chunk is the stationary
                # operand so the output lands batch-major.
                for jg in range(6):
                    tensor.wait_ge(s_tanh, jg + 2)
                    mm = nc.tensor.matmul(
                        lps[:, :T_OUT],
                        hT_sb[:, jg * LOCAL_B : (jg + 1) * LOCAL_B],
                        cwT_t[:, jg * T_OUT : (jg + 1) * T_OUT],
                        start=(jg == 0), stop=(jg == 5),
                    )
                mm.then_inc(s_cls, 1)

            @block.vector
            def _(vector):
                vector.wait_ge(s_smalls, 48)
                vector.wait_ge(s_pool, 1)
                nc.vector.tensor_scalar_mul(
                    pooled_sb[:, 0 : 256 if sc3 else 512],
                    pooled_a[:, 0 : 256 if sc3 else 512], invl_ap
                ).then_inc(s_scA, 1)
                for c in range(6):
                    vector.wait_ge(s_tr, c + 1)
                    nc.vector.tensor_copy(
                        pooledT_sb[:, c * LOCAL_B : (c + 1) * LOCAL_B],
                        tp[c % 3][:, :LOCAL_B],
                    ).then_inc(s_ptcopy, 1)
                vector.wait_ge(s_cls, 1)
                nc.vector.tensor_add(
                    logits_sb[:], lps[:, :T_OUT], clsb_t[:]
                ).then_inc(s_log, 1)

    nc.compile()
    return nc


def _quantize_feedback_f8(hs, lens):
    """e4m3 quantization with per-(sample, channel) error feedback along the
    row sequence, plus a final mop-up row holding the residual. The device sums
    rows, so per-row quantization errors telescope: the pooled sum sees only
    the (quantized) final residual."""
    f32 = np.float32
    q = np.zeros((B, S, H), NP_F8)
    err = np.zeros((B, H), f32)
    maxlen = int(lens.max())
    hsf = hs.astype(f32, copy=False)
    for s in range(1, maxlen + 1):
        active = lens >= s
        x = hsf[active, s, :] + err[active]
        qs = x.astype(NP_F8)
        q[active, s, :] = qs
        err[active] = x - qs.astype(f32)
    mop = err.astype(NP_F8)
    return q, mop


def _kernel_f8(hs, lens, dense_w, dense_b, cls_w, cls_b):
    global last_results
    # ---- assign samples to cores: greedy LPT on (len+1) with a 32-per-core cap
    w = lens + 1  # +1 for the mop-up row
    order = np.argsort(-w, kind="stable")
    core_samples = [[] for _ in range(N_CORES)]
    load = np.zeros(N_CORES, dtype=np.int64)
    for b in order:
        open_cores = [c for c in range(N_CORES) if len(core_samples[c]) < LOCAL_B]
        c = min(open_cores, key=lambda c: load[c])
        core_samples[c].append(int(b))
        load[c] += int(w[b])
    max_rows = int(load.max())
    n_full, rem = divmod(max_rows, 8 * P8)
    g_last = -(-rem // P8)
    g_last += g_last % 2  # DoubleRow consumes subtile pairs
    if g_last == 8:
        n_full, g_last = n_full + 1, 0
    n_groups = n_full + (1 if g_last else 0)
    NR = 8 * P8 * n_full + P8 * g_last
    K_cols = 8 * n_full + g_last

    key = ("f8", n_full, g_last, WIDE, TAPER, SC3)
    if key not in _cache:
        _cache[key] = _build_program_f8(
            n_full, g_last, bool(WIDE), bool(TAPER), bool(SC3)
        )
    nc = _cache[key]

    q, mop = _quantize_feedback_f8(hs, lens)

    # ---- shared (replicated) head tensors, fp16
    dwT_host = np.empty((128, 6 * H), np.float32)
    for c in range(6):
        dwT_host[:, c * H : (c + 1) * H] = dense_w[:, c * 128 : (c + 1) * 128].T
    cwT_host = np.empty((128, 6 * T_OUT), np.float32)
    for jg in range(6):
        cwT_host[:, jg * T_OUT : (jg + 1) * T_OUT] = cls_w[:, jg * 128 : (jg + 1) * 128].T
    dwT_host = dwT_host.astype(np.float16)
    cwT_host = cwT_host.astype(np.float16)
    ident2_host = np.eye(32, dtype=np.float16)
    db6_host = np.ascontiguousarray(dense_b.reshape(6, 128).T)
    clsb_host = np.ascontiguousarray(
        np.broadcast_to(cls_b, (LOCAL_B, T_OUT)).astype(np.float32)
    )

    q2 = q.reshape(B * S, H)
    in_maps = []
    for c in range(N_CORES):
        samples = core_samples[c]
        lens_c = lens[samples]
        packed = np.zeros((NR, H), NP_F8)
        pos = 0
        for b in samples:
            L = int(lens[b])
            packed[pos : pos + L] = q2[b * S + 1 : b * S + 1 + L]
            packed[pos + L] = mop[b]
            pos += L + 1

        # membership: row j -> (group t, partition p, subtile q); G_t rows are
        # consecutive per partition within a group.
        j = np.arange(pos)
        t = np.minimum(j // (8 * P8), n_groups - 1)
        j2 = j - t * (8 * P8)
        g_t = np.where(t < n_full, 8, g_last)
        p = j2 // g_t
        qsub = j2 - p * g_t
        kcol = 8 * t + qsub
        local_b = np.repeat(np.arange(LOCAL_B), lens_c + 1)
        mem = np.zeros((P8, K_cols * LOCAL_B), NP_F8)
        mem[p, kcol * LOCAL_B + local_b] = NP_F8(1.0)

        smalls = np.zeros((128, 8), np.float32)
        smalls[:LOCAL_B, 0] = 1.0 / lens_c.astype(np.float32)
        smalls[:, 2:8] = db6_host
        im = {
            "member": mem,
            "dwT": dwT_host,
            "cwT": cwT_host,
            "ident2": ident2_host,
            "smalls": smalls,
            "clsb": clsb_host,
        }
        # sdata layout [P8, K_cols, H]: row j -> sdata[p_j, kcol_j, :]
        arr = np.zeros((P8, K_cols, H), NP_F8)
        arr[p, kcol] = packed[:pos]
        im["hs"] = arr.reshape(P8, K_cols * H)
        in_maps.append(im)

    trace = bool(os.environ.get("KERNEL_TRACE"))
    try:
        res = bass_utils.run_bass_kernel_spmd(
            nc, in_maps, list(range(N_CORES)), trace=trace
        )
    except Exception:
        res = bass_utils.run_bass_kernel_spmd(
            nc, in_maps, list(range(N_CORES)), trace=trace
        )
    last_results = res

    logits = np.zeros((B, T_OUT), np.float32)
    for c in range(N_CORES):
        logits[core_samples[c], :] = res.results[c]["out"]
    return logits


def _build_program_raw(T_g: int, mode: str, p_last: int = 128) -> bass.Bass:
    """Hand-synchronized variant: one FIFO HWDGE ring delivers member, the
    stream tiles (in consumption order), then the head weights; each engine's
    program carries explicit sem waits. PSUM is budgeted bank-by-bank:
    pooled_a, pooled_b, tp0-2, hps0-1, lps = 8 banks."""
    sdt = mybir.dt.float16 if mode == "f16" else mybir.dt.bfloat16
    n_streams = 2 if mode == "f32x2" else 1
    # Head dtype: fp16/bf16 single-stream modes run the whole head in the
    # stream dtype (fp32 head matmuls cost 2 LDWEIGHTS+MATMUL passes each —
    # measured ~17us for the 48 head matmuls vs ~5us in fp16). The f32x2
    # accuracy mode keeps the head in fp32.
    hdt = F32 if n_streams == 2 else sdt
    W = G * H

    # No collectives -> no partition id; skipping it drops 5 per-engine
    # TENSOR_LOADs (~2us) from the launch preamble.
    nc = bacc.Bacc(enable_partition_id=False)
    streams_d = [
        nc.declare_dram_parameter(f"hs{i}", [T_g, 128, W], sdt, isOutput=False)
        for i in range(n_streams)
    ]
    member_d = nc.declare_dram_parameter(
        "member", [128, T_g * G * LOCAL_B], sdt, isOutput=False
    )
    dwT_d = nc.declare_dram_parameter("dwT", [128, 6 * H], hdt, isOutput=False)
    cwT_d = nc.declare_dram_parameter("cwT", [128, 6 * T_OUT], hdt, isOutput=False)
    ident2_d = nc.declare_dram_parameter("ident2", [32, 32], hdt, isOutput=False)
    # smalls blob [128, 40] f32: col 0 invl (rows 0-31), col 1 cls_b (rows
    # 0-95), cols 2-7 dense_b chunks, cols 8-39 identity (rows 0-31).
    smalls_d = nc.declare_dram_parameter("smalls", [128, 40], F32, isOutput=False)
    # cls_b pre-broadcast to [32, 96] on the host: lets the classifier output
    # land as logits [b, t] (32 descriptors x 384B on the store instead of 96
    # x 128B — the store's tail rides the slowest SDMA engine).
    clsb_d = nc.declare_dram_parameter("clsb", [LOCAL_B, T_OUT], F32, isOutput=False)
    out_d = nc.declare_dram_parameter("out", [LOCAL_B, T_OUT], F32, isOutput=True)

    with ExitStack() as ctx:
        member_t = ctx.enter_context(
            nc.sbuf_tensor([128, T_g * G * LOCAL_B], sdt)
        )
        stile = [
            [
                ctx.enter_context(nc.sbuf_tensor(f"stile{si}_{t}", [128, W], sdt))
                for t in range(T_g)
            ]
            for si in range(n_streams)
        ]
        smalls_t = ctx.enter_context(nc.sbuf_tensor([128, 40], F32))
        ident2_t = ctx.enter_context(nc.sbuf_tensor([32, 32], hdt))
        dwT_t = ctx.enter_context(nc.sbuf_tensor([128, 6 * H], hdt))
        cwT_t = ctx.enter_context(nc.sbuf_tensor([128, 6 * T_OUT], hdt))
        pooled_sb = ctx.enter_context(nc.sbuf_tensor([LOCAL_B, H], hdt))
        pooledT_sb = ctx.enter_context(nc.sbuf_tensor([128, 6 * LOCAL_B], hdt))
        hT_sb = ctx.enter_context(nc.sbuf_tensor([128, 6 * LOCAL_B], hdt))
        clsb_t = ctx.enter_context(nc.sbuf_tensor([LOCAL_B, T_OUT], F32))
        logits_sb = ctx.enter_context(nc.sbuf_tensor([LOCAL_B, T_OUT], F32))
        warm_sb = ctx.enter_context(nc.sbuf_tensor([128, 512], sdt))

        pooled_a = ctx.enter_context(nc.psum_tensor([LOCAL_B, 512], F32))
        pooled_b = ctx.enter_context(nc.psum_tensor([LOCAL_B, 512], F32))
        tp = [
            ctx.enter_context(nc.psum_tensor(f"tp{i}", [128, 512], hdt))
            for i in range(3)
        ]
        hps = [
            ctx.enter_context(nc.psum_tensor(f"hps{i}", [128, 512], F32))
            for i in range(2)
        ]
        lps = ctx.enter_context(nc.psum_tensor([LOCAL_B, 512], F32))

        invl_ap = smalls_t[:LOCAL_B, 0:1]
        db6_ap = smalls_t[:, 2:8]

        # Single-stream modes: each stream tile arrives as two half-DMAs with
        # their own sems. The matmuls for the first half run while the second
        # half transfers — and when a slow SDMA engine dribbles the ring
        # tail, only the last half-tile's 8 matmuls wait on it.
        halved = n_streams == 1
        s_member = nc.alloc_semaphore("s_member")
        s_stream = [nc.alloc_semaphore(f"s_stream{t}") for t in range(T_g)]
        s_streamB = [nc.alloc_semaphore(f"s_streamB{t}") for t in range(T_g)]
        s_smalls = nc.alloc_semaphore("s_smalls")
        s_hw = nc.alloc_semaphore("s_hw")  # dwT+cwT (adjacent on one ring:
        # a full-count wait of 32 implies both transfers complete)
        s_pool = nc.alloc_semaphore("s_pool")
        s_scaled = nc.alloc_semaphore("s_scaled")
        s_tr = nc.alloc_semaphore("s_tr")
        s_ptcopy = nc.alloc_semaphore("s_ptcopy")
        s_head = nc.alloc_semaphore("s_head")
        s_tanh = nc.alloc_semaphore("s_tanh")
        s_cls = nc.alloc_semaphore("s_cls")
        s_log = nc.alloc_semaphore("s_log")
        s_out = nc.alloc_semaphore("s_out")
        s_warm = nc.alloc_semaphore("s_warm")

        with nc.Block() as block:

            @block.gpsimd
            def _(gpsimd):
                nc.gpsimd.memset(warm_sb[:], 0.0).then_inc(s_warm, 1)
                # The last group holds only the load-balance remainder: under
                # the p-major packing its real rows occupy partitions
                # [0, p_last). Those above are never transferred (the DMA
                # below skips them) — zero once so the matmuls read 0s
                # (membership is 0 there, but fp16 garbage could be NaN).
                if p_last < 128:
                    for si in range(n_streams):
                        nc.gpsimd.memset(
                            stile[si][T_g - 1][p_last:, :], 0.0
                        ).then_inc(s_warm, 1)

            @block.sync
            def _(sync):
                # FIFO ring: group-0 slice of member first, then stream tile
                # 0, then the rest in consumption order — the first matmul
                # needs only the first two transfers (~1.6MB).
                g0 = G * LOCAL_B
                sync.dma_start(
                    out=member_t[:, :g0], in_=member_d[:, :g0]
                ).then_inc(s_member, 16)
                def stream_tile(t):
                    pl = p_last if t == T_g - 1 else 128
                    if halved:
                        hw2 = W // 2
                        sync.dma_start(
                            out=stile[0][t][:pl, :hw2],
                            in_=streams_d[0][t][:pl, :hw2],
                        ).then_inc(s_stream[t], 16)
                        sync.dma_start(
                            out=stile[0][t][:pl, hw2:],
                            in_=streams_d[0][t][:pl, hw2:],
                        ).then_inc(s_streamB[t], 16)
                    else:
                        for si in range(n_streams):
                            sync.dma_start(
                                out=stile[si][t][:pl, :], in_=streams_d[si][t][:pl, :]
                            ).then_inc(s_stream[t], 16)

                stream_tile(0)
                sync.dma_start(
                    out=member_t[:, g0:], in_=member_d[:, g0:]
                ).then_inc(s_member1, 16)
                for t in range(1, T_g):
                    stream_tile(t)
                    if t == 2:
                        # tiny consts ride mid-ring: early enough that their
                        # last descriptors never trail on the slowest SDMA
                        # engine (which stalls the head), late enough that
                        # their issue slots don't delay the ramp-phase
                        # transfers the first matmuls wait on.
                        sync.dma_start(
                            out=smalls_t[:], in_=smalls_d[:]
                        ).then_inc(s_smalls, 16)
                        sync.dma_start(
                            out=ident2_t[:], in_=ident2_d[:]
                        ).then_inc(s_smalls, 16)
                        sync.dma_start(out=clsb_t[:], in_=clsb_d[:]).then_inc(
                            s_smalls, 16
                        )
                sync.dma_start(out=dwT_t[:], in_=dwT_d[:]).then_inc(s_hw, 16)
                sync.dma_start(out=cwT_t[:], in_=cwT_d[:]).then_inc(s_hw, 16)
                # output store (waits for the DVE bias-add)
                sync.wait_ge(s_log, 1)
                sync.dma_start(out=out_d[:], in_=logits_sb[:]).then_inc(s_out, 16)
                sync.wait_ge(s_out, 16)

            @block.tensor
            def _(tensor):
                # HAM management: the PE clock-gate re-throttles to 4/8 after
                # an idle activity window (~3.4us). The kernel is DMA-paced,
                # so the PE would idle ~1us per group — enough, with bad
                # window phase, to oscillate between 1.2 and 2.4 GHz. Filler
                # matmuls on a zeroed scratch tile (into the hps bank, which
                # the dense phase later resets with start=True) keep the PE
                # busy through every wait.
                def filler(n):
                    for _ in range(n):
                        nc.tensor.matmul(
                            hps[0][:, :512],
                            warm_sb[:, :128],
                            warm_sb[:, :512],
                            start=True, stop=True,
                        )

                tensor.wait_ge(s_warm, 1 + (n_streams if p_last < 128 else 0))
                filler(12)
                n_mm = T_g * G * n_streams
                i_mm = 0
                last_mm = None
                for t in range(T_g):
                    if t == 0:
                        tensor.wait_ge(s_member, 16)
                    else:
                        filler(4)
                        tensor.wait_ge(s_member1, 16)
                    for q in range(G):
                        if q == 0:
                            tensor.wait_ge(s_stream[t], 16 * n_streams)
                        elif halved and q == G // 2:
                            tensor.wait_ge(s_streamB[t], 16)
                        k = t * G + q
                        lhsT = member_t[:, k * LOCAL_B : (k + 1) * LOCAL_B]
                        for si in range(n_streams):
                            st = stile[si][t]
                            first, last = i_mm == 0, i_mm == n_mm - 1
                            nc.tensor.matmul(
                                pooled_a[:, :512], lhsT,
                                st[:, q * H : q * H + 512],
                                start=first, stop=last,
                            )
                            last_mm = nc.tensor.matmul(
                                pooled_b[:, : H - 512], lhsT,
                                st[:, q * H + 512 : (q + 1) * H],
                                start=first, stop=last,
                            )
                            i_mm += 1
                last_mm.then_inc(s_pool, 1)
                # transposes (need DVE scale + the identity matrix)
                filler(3)
                tensor.wait_ge(s_smalls, 48)
                for c in range(6):
                    tensor.wait_ge(s_scaled, 1 if c < 4 else 2)
                    if c >= 3:
                        tensor.wait_ge(s_ptcopy, c - 2)
                    nc.tensor.transpose(
                        tp[c % 3][:, :LOCAL_B],
                        pooled_sb[:, c * 128 : (c + 1) * 128],
                        ident2_t[:],
                    ).then_inc(s_tr, 1)
                # dense layer
                filler(2)
                tensor.wait_ge(s_ptcopy, 6)
                tensor.wait_ge(s_hw, 32)
                for jg in range(6):
                    if jg >= 2:
                        tensor.wait_ge(s_tanh, jg - 1)
                    for c in range(6):
                        mm = nc.tensor.matmul(
                            hps[jg % 2][:, :LOCAL_B],
                            dwT_t[:, c * H + jg * 128 : c * H + (jg + 1) * 128],
                            pooledT_sb[:, c * LOCAL_B : (c + 1) * LOCAL_B],
                            start=(c == 0), stop=(c == 5),
                        )
                    mm.then_inc(s_head, 1)
                # classifier
                # classifier: logits[b, t] — hT chunk is the stationary
                # operand so the output lands batch-major.
                tensor.wait_ge(s_cwT, 16)
                for jg in range(6):
                    tensor.wait_ge(s_tanh, jg + 1)
                    mm = nc.tensor.matmul(
                        lps[:, :T_OUT],
                        hT_sb[:, jg * LOCAL_B : (jg + 1) * LOCAL_B],
                        cwT_t[:, jg * T_OUT : (jg + 1) * T_OUT],
                        start=(jg == 0), stop=(jg == 5),
                    )
                mm.then_inc(s_cls, 1)

            @block.vector
            def _(vector):
                vector.wait_ge(s_smalls, 48)
                vector.wait_ge(s_pool, 1)
                nc.vector.tensor_scalar_mul(
                    pooled_sb[:, 0:512], pooled_a[:, :512], invl_ap
                ).then_inc(s_scaled, 1)
                nc.vector.tensor_scalar_mul(
                    pooled_sb[:, 512:H], pooled_b[:, : H - 512], invl_ap
                ).then_inc(s_scaled, 1)
                for c in range(6):
                    vector.wait_ge(s_tr, c + 1)
                    nc.vector.tensor_copy(
                        pooledT_sb[:, c * LOCAL_B : (c + 1) * LOCAL_B],
                        tp[c % 3][:, :LOCAL_B],
                    ).then_inc(s_ptcopy, 1)
                vector.wait_ge(s_cls, 1)
                nc.vector.tensor_add(
                    logits_sb[:], lps[:, :T_OUT], clsb_t[:]
                ).then_inc(s_log, 1)

            @block.scalar
            def _(scalar):
                scalar.wait_ge(s_smalls, 48)
                for jg in range(6):
                    scalar.wait_ge(s_head, jg + 1)
                    nc.scalar.activation(
                        hT_sb[:, jg * LOCAL_B : (jg + 1) * LOCAL_B],
                        hps[jg % 2][:, :LOCAL_B],
                        mybir.ActivationFunctionType.Tanh,
                        bias=db6_ap[:, jg : jg + 1],
                    ).then_inc(s_tanh, 1)

    nc.compile()
    return nc


def kernel(hidden_states, pivot_len_list, dense_w, dense_b, cls_w, cls_b):
    global last_results
    hs = np.ascontiguousarray(np.asarray(hidden_states, dtype=np.float32))
    lens = np.asarray(pivot_len_list).astype(np.int64)
    dense_w = np.asarray(dense_w, dtype=np.float32)
    dense_b = np.asarray(dense_b, dtype=np.float32)
    cls_w = np.asarray(cls_w, dtype=np.float32)
    cls_b = np.asarray(cls_b, dtype=np.float32)
    assert hs.shape == (B, S, H), hs.shape
    assert lens.shape == (B,), lens.shape

    if MODE == "f8":
        return _kernel_f8(hs, lens, dense_w, dense_b, cls_w, cls_b)

    mode = MODE
    np_sdt = np.float16 if mode == "f16" else ml_dtypes.bfloat16

    # ---- assign samples to cores: greedy LPT with a hard 32-per-core cap
    order = np.argsort(-lens, kind="stable")
    core_samples = [[] for _ in range(N_CORES)]
    load = np.zeros(N_CORES, dtype=np.int64)
    for b in order:
        open_cores = [c for c in range(N_CORES) if len(core_samples[c]) < LOCAL_B]
        c = min(open_cores, key=lambda c: load[c])
        core_samples[c].append(int(b))
        load[c] += int(lens[b])
    T_g = max(1, -(-int(load.max()) // ROWS_PER_GROUP))

    impl = IMPL
    # Partitions actually occupied in the final (remainder) group under the
    # p-major packing; the rest of that tile is padding and never transferred.
    rows_last = int(load.max()) - (T_g - 1) * ROWS_PER_GROUP
    # partition offsets must be 32-aligned (engine base-partition constraint)
    p_last = min(128, max(32, 32 * -(--(-rows_last // G) // 32)))
    key = (T_g, mode, impl, p_last)
    if key not in _cache:
        if impl == "raw":
            _cache[key] = _build_program_raw(T_g, mode, p_last)
        else:
            _cache[key] = _build_program(T_g, mode)
    nc = _cache[key]

    # ---- shared (replicated) head tensors
    dwT_host = np.empty((128, 6 * H), np.float32)
    for c in range(6):
        dwT_host[:, c * H : (c + 1) * H] = dense_w[:, c * 128 : (c + 1) * 128].T
    cwT_host = np.empty((128, 6 * T_OUT), np.float32)
    for jg in range(6):
        cwT_host[:, jg * T_OUT : (jg + 1) * T_OUT] = cls_w[:, jg * 128 : (jg + 1) * 128].T
    db6_host = np.ascontiguousarray(dense_b.reshape(6, 128).T)
    cb1_host = np.ascontiguousarray(cls_b.reshape(T_OUT, 1))
    ident_host = np.eye(32, dtype=np.float32)

    # ---- per-core packing
    hs2 = hs.reshape(B * S, H)
    NR = T_g * ROWS_PER_GROUP
    in_maps = []
    for c in range(N_CORES):
        samples = core_samples[c]
        lens_c = lens[samples]
        idx = np.concatenate(
            [np.arange(b * S + 1, b * S + 1 + lens[b]) for b in samples]
        )
        n = idx.size
        packed = np.zeros((NR, H), np.float32)
        packed[:n] = hs2[idx]
        if mode == "f32x2":
            hi = packed.astype(ml_dtypes.bfloat16)
            lo = (packed - hi.astype(np.float32)).astype(ml_dtypes.bfloat16)
            stream_arrays = [hi, lo]
        else:
            stream_arrays = [packed.astype(np_sdt)]

        j = np.arange(n)
        tt = j // ROWS_PER_GROUP
        p = (j % ROWS_PER_GROUP) // G
        q = j % G
        local_b = np.repeat(np.arange(LOCAL_B), lens_c)
        mem = np.zeros((128, T_g * G * LOCAL_B), np_sdt)
        mem[p, (tt * G + q) * LOCAL_B + local_b] = np_sdt(1.0)

        invl_host = (1.0 / lens_c.astype(np.float32)).reshape(LOCAL_B, 1)
        im = {"member": mem, "dwT": dwT_host, "cwT": cwT_host}
        if impl == "raw":
            np_hdt = np.float32 if mode == "f32x2" else np_sdt
            im["dwT"] = dwT_host.astype(np_hdt)
            im["cwT"] = cwT_host.astype(np_hdt)
            im["ident2"] = np.eye(32, dtype=np.float32).astype(np_hdt)
            im["clsb"] = np.ascontiguousarray(
                np.broadcast_to(cls_b, (LOCAL_B, T_OUT)).astype(np.float32)
            )
            smalls = np.zeros((128, 40), np.float32)
            smalls[:LOCAL_B, 0:1] = invl_host
            smalls[:T_OUT, 1:2] = cb1_host
            smalls[:, 2:8] = db6_host
            smalls[:32, 8:40] = ident_host
            im["smalls"] = smalls
        else:
            im["db6"] = db6_host
            im["cb1"] = cb1_host
            im["invl"] = invl_host
            im["ident"] = ident_host
        for i, arr in enumerate(stream_arrays):
            im[f"hs{i}"] = arr.reshape(T_g, 128, G * H)
        in_maps.append(im)

    trace = bool(os.environ.get("KERNEL_TRACE"))
    try:
        res = bass_utils.run_bass_kernel_spmd(
            nc, in_maps, list(range(N_CORES)), trace=trace
        )
    except Exception:
        # Transient NRT device errors (e.g. NRT_EXEC_UNIT_UNRECOVERABLE after
        # many back-to-back launches) clear on retry.
        res = bass_utils.run_bass_kernel_spmd(
            nc, in_maps, list(range(N_CORES)), trace=trace
        )
    last_results = res

    logits = np.zeros((B, T_OUT), np.float32)
    for c in range(N_CORES):
        o = res.results[c]["out"]
        logits[core_samples[c], :] = o if impl == "raw" else o.T
    return logits



# revision 34
# speedup vs baseline: 1.1191x; 1.0276x over previous
"""Trainium2 Bass kernel: ragged mean-pool over [1, len_i] + Linear->tanh->Linear head.

Strategy (pure data parallel over batch, 8 NeuronCores):
  * Host: balance the 256 samples across 8 cores (32 each) by total row count,
    gather only the needed rows hidden_states[b, 1:len_b+1, :] into a packed
    dense array per core (the ragged/masked structure becomes a small 0/1
    "membership" matrix), and encode rows compactly (fp16 by default).
  * Device: stream packed row-tiles [128, 8*768]; for each 128-row subtile do
    pooled[b,h] += member[r,b] * rows[r,h] as a PE matmul with the membership
    matrix as the stationary operand, accumulating all tiles into one PSUM
    region. Then scale by 1/len, transpose, and run the tiny 768x768 tanh head
    and 96x768 classifier fully on-chip. Output is logits^T [96, 32] per core.
  * Host: scatter per-core outputs back to the full [256, 96] logits.

The compiled program depends only on (T_g, mode) where T_g = number of
1024-row groups per core -- not on the actual lengths -- so recompiles are
rare. All raggedness lives in data (packing + membership).
"""

import os
from contextlib import ExitStack

import numpy as np
import ml_dtypes

import concourse.bass as bass
import concourse.mybir as mybir
from concourse import bacc, bass_utils, tile

B, S, H, T_OUT = 256, 512, 768, 96
N_CORES = 8
LOCAL_B = B // N_CORES        # 32 samples per core
G = 8                         # packed rows per partition per group tile
ROWS_PER_GROUP = 128 * G      # 1024
F32 = mybir.dt.float32

# Row encodings: "f8" (1B/elem e4m3 + per-sample error-feedback + one mop-up
# residual row, ~6e-4 rel err, DoubleRow double-pumped PE), "f16" (2B/elem,
# ~2e-4 rel err), "f32x2" (bf16 hi+lo pair, 4B/elem, ~2e-6 rel err), "bf16"
# (2B/elem, ~1.4e-3 rel err).
MODE = os.environ.get("KERNEL_MODE", "f8")
# f8 pooling matmul width: 1 = double-wide DoubleRow (rhs free 1024, 2 matmuls
# per subtile pair), 0 = 3 matmuls of 512. Settable per-call via kernel.WIDE.
WIDE = int(os.environ.get("KERNEL_WIDE", "1"))
# f8 pooled-scale split (DVE [0:256] + ACT [256:512],[512:768]) HANGS the
# device -- suspected ACT+DVE concurrent read of the same PSUM bank. Hard
# disabled; the incumbent keeps the two engines on separate pooled banks.
SC3 = 0
# f8 stream tail chunking: 1 = taper the last chunks ([6,4,2] subtiles), 0 =
# uniform 8-subtile chunks. Settable per-call via kernel.TAPER.
TAPER = int(os.environ.get("KERNEL_TAPER", "0"))
# 1 = early chunk pairs share one semaphore (full-count 32 wait: safe because
# each SDMA engine is FIFO within the ring, so value 32 implies both DMAs
# fully landed) -- fewer sems shortens the epilogue reset chain.
PAIRSEM = int(os.environ.get("KERNEL_PAIRSEM", "0"))
# "raw" = hand-synchronized Bacc program (no Tile scheduler, minimal
# semaphore traffic and no kernel-tail sem-reset butterfly); "tile" = the
# TileContext-scheduled variant.
IMPL = os.environ.get("KERNEL_IMPL", "raw")

_cache: dict = {}
last_results = None  # BassKernelResults of the most recent run (for test.py)


def _build_program(T_g: int, mode: str) -> bass.Bass:
    sdt = mybir.dt.float16 if mode == "f16" else mybir.dt.bfloat16
    n_streams = 2 if mode == "f32x2" else 1
    W = G * H  # free-dim width of a group tile

    # Bacc (not raw Bass): its compile() pass splits multi-semaphore waits
    # into EventSemaphore chains — hardware allows at most 1 wait per
    # instruction — and moves matmul waits onto ldweights.
    nc = bacc.Bacc()
    streams = [
        nc.declare_dram_parameter(f"hs{i}", [T_g, 128, W], sdt, isOutput=False)
        for i in range(n_streams)
    ]
    member = nc.declare_dram_parameter(
        "member", [128, T_g * G * LOCAL_B], sdt, isOutput=False
    )
    dwT = nc.declare_dram_parameter("dwT", [128, 6 * H], F32, isOutput=False)
    cwT = nc.declare_dram_parameter("cwT", [128, 6 * T_OUT], F32, isOutput=False)
    db6 = nc.declare_dram_parameter("db6", [128, 6], F32, isOutput=False)
    cb1 = nc.declare_dram_parameter("cb1", [T_OUT, 1], F32, isOutput=False)
    invl = nc.declare_dram_parameter("invl", [LOCAL_B, 1], F32, isOutput=False)
    ident = nc.declare_dram_parameter("ident", [32, 32], F32, isOutput=False)
    out = nc.declare_dram_parameter("out", [T_OUT, LOCAL_B], F32, isOutput=True)

    with ExitStack() as ctx:
        tc = ctx.enter_context(tile.TileContext(nc))
        const_pool = ctx.enter_context(tc.tile_pool(name="const", bufs=1))
        # All group tiles resident at once (single-stream modes fit: T_g * 12KB
        # per partition). Slot reuse would attach 3 semaphore waits to the
        # reload DMAs, which the DMA instruction encoding cannot carry.
        in_bufs = T_g * n_streams if n_streams == 1 else 3
        in_pool = ctx.enter_context(tc.tile_pool(name="inp", bufs=in_bufs))
        sb_pool = ctx.enter_context(tc.tile_pool(name="sb", bufs=1))
        ps_pooled = ctx.enter_context(tc.tile_pool(name="psp", bufs=1, space="PSUM"))
        ps_small = ctx.enter_context(tc.tile_pool(name="pss", bufs=2, space="PSUM"))

        # DMA order matters: queues drain in emission order, and the pooling
        # matmuls only need `member` + their stream tile. Load those first;
        # the head weights (dwT/cwT, ~2.7MB) are consumed only after all
        # pooling, so they stream in behind and overlap the pooling phase.
        member_t = const_pool.tile([128, T_g * G * LOCAL_B], sdt)
        nc.sync.dma_start(member_t[:], member[:])

        all_stiles = []
        for t in range(T_g):
            stiles = []
            for si, s in enumerate(streams):
                st = in_pool.tile([128, W], sdt, tag=f"s{si}")
                nc.sync.dma_start(st[:], s[t])
                stiles.append(st)
            all_stiles.append(stiles)

        invl_t = const_pool.tile([LOCAL_B, 1], F32)
        nc.sync.dma_start(invl_t[:], invl[:])
        ident_t = const_pool.tile([32, 32], F32)
        nc.sync.dma_start(ident_t[:], ident[:])
        dwT_t = const_pool.tile([128, 6 * H], F32)
        nc.sync.dma_start(dwT_t[:], dwT[:])
        cwT_t = const_pool.tile([128, 6 * T_OUT], F32)
        nc.sync.dma_start(cwT_t[:], cwT[:])
        db6_t = const_pool.tile([128, 6], F32)
        nc.sync.dma_start(db6_t[:], db6[:])
        cb1_t = const_pool.tile([T_OUT, 1], F32)
        nc.sync.dma_start(cb1_t[:], cb1[:])

        # Pre-touch small const tiles on the engine that will consume them:
        # several ISA instruction encodings carry only ONE semaphore wait, so
        # the consuming op must not need both its data-producer wait and a
        # const-DMA wait. Touching the const here advances that engine's
        # observed clock past the const DMA, and the later wait is elided.
        scratch = const_pool.tile([128, 8], F32)
        nc.vector.tensor_copy(scratch[:LOCAL_B, 0:1], invl_t[:])
        nc.vector.tensor_copy(scratch[:T_OUT, 1:2], cb1_t[:])
        nc.scalar.activation(
            scratch[:, 2:8], db6_t[:], mybir.ActivationFunctionType.Copy
        )

        # ---- ragged pooling: pooled[b, h] = sum over packed rows r of
        #      member[r, b] * row[r, h], accumulated in PSUM over all tiles.
        pooled_a = ps_pooled.tile([LOCAL_B, 512], F32, tag="pa")
        pooled_b = ps_pooled.tile([LOCAL_B, H - 512], F32, tag="pb")
        n_mm = T_g * G * n_streams  # matmuls per PSUM region
        i_mm = 0
        for t in range(T_g):
            stiles = all_stiles[t]
            for q in range(G):
                k = t * G + q
                lhsT = member_t[:, k * LOCAL_B : (k + 1) * LOCAL_B]
                for st in stiles:
                    first, last = i_mm == 0, i_mm == n_mm - 1
                    nc.tensor.matmul(
                        pooled_a[:], lhsT, st[:, q * H : q * H + 512],
                        start=first, stop=last,
                    )
                    nc.tensor.matmul(
                        pooled_b[:], lhsT, st[:, q * H + 512 : (q + 1) * H],
                        start=first, stop=last,
                    )
                    i_mm += 1

        # ---- mean: scale each sample's partition by 1/len
        pooled_sb = sb_pool.tile([LOCAL_B, H], F32)
        nc.vector.tensor_scalar_mul(pooled_sb[:, 0:512], pooled_a[:], invl_t[:])
        nc.vector.tensor_scalar_mul(pooled_sb[:, 512:H], pooled_b[:], invl_t[:])

        # ---- transpose pooled [32, 768] -> pooledT [768, 32] via PE
        pooledT_sb = sb_pool.tile([128, 6 * LOCAL_B], F32)
        for c in range(6):
            tp = ps_small.tile([128, LOCAL_B], F32, tag="tp")
            nc.tensor.transpose(
                tp[:], pooled_sb[:, c * 128 : (c + 1) * 128], ident_t[:]
            )
            nc.vector.tensor_copy(pooledT_sb[:, c * LOCAL_B : (c + 1) * LOCAL_B], tp[:])

        # ---- dense layer + tanh: hT[j, b] = tanh(dense_b[j] + sum_h dwT[h, j] pooledT[h, b])
        hT_sb = sb_pool.tile([128, 6 * LOCAL_B], F32)
        for jg in range(6):
            hps = ps_small.tile([128, LOCAL_B], F32, tag="hps")
            for c in range(6):
                nc.tensor.matmul(
                    hps[:],
                    dwT_t[:, c * H + jg * 128 : c * H + (jg + 1) * 128],
                    pooledT_sb[:, c * LOCAL_B : (c + 1) * LOCAL_B],
                    start=(c == 0), stop=(c == 5),
                )
            nc.scalar.activation(
                hT_sb[:, jg * LOCAL_B : (jg + 1) * LOCAL_B],
                hps[:],
                mybir.ActivationFunctionType.Tanh,
                bias=db6_t[:, jg : jg + 1],
            )

        # ---- classifier: logitsT[t, b] = cls_b[t] + sum_j cwT[j, t] hT[j, b]
        lps = ps_small.tile([T_OUT, LOCAL_B], F32, tag="lps")
        for jg in range(6):
            nc.tensor.matmul(
                lps[:],
                cwT_t[:, jg * T_OUT : (jg + 1) * T_OUT],
                hT_sb[:, jg * LOCAL_B : (jg + 1) * LOCAL_B],
                start=(jg == 0), stop=(jg == 5),
            )
        logits_sb = sb_pool.tile([T_OUT, LOCAL_B], F32)
        nc.vector.tensor_scalar_add(logits_sb[:], lps[:], cb1_t[:])
        # SWDGE store: lands on a fresh DMASW sem lane, so it carries only the
        # DVE wait (every encoding has a single wait slot).
        nc.gpsimd.dma_start(out[:], logits_sb[:])

    nc.compile()
    return nc


F8 = mybir.dt.float8e4
P8 = 128                      # stream partitions in f8 mode (124 was tried to
                              # starve slow SDMA engine 15, but non-128
                              # partition DMAs fall off the fast descriptor
                              # path: 2x slower overall)
NP_F8 = ml_dtypes.float8_e4m3  # IEEE-style e4m3 (max 240) == TRN FP8_EXP4
F16 = mybir.dt.float16


def _build_program_f8(
    n_full: int, g_last: int, wide: bool = False, taper: bool = False,
    sc3: bool = False, pairsem: bool = False,
) -> bass.Bass:
    """fp8 variant: e4m3 streams + membership, DoubleRow double-pumped pooling
    matmuls (256-row contraction per instruction), fp16 head.

    Layout: one flat stream tensor sdata [128, K_cols, H] (K_cols 128-row
    subtiles; groups of 8 subtiles share a membership accumulation pattern,
    the optional last group holds g_last even subtiles). All padding rows are
    host-zeroed (fp8 0x00) so there are no device memsets and no gpsimd use.

    DMA: the stream rides the Sync HWDGE ring in a few LARGE chunks (small
    first chunk for a fast pipeline start) -- per-instruction descriptor
    generation (~0.7-1.3us) otherwise cannot keep 16 SDMA engines fed at
    ~400GB/s with halved (fp8) per-instruction bytes. Everything else
    (member, smalls, head weights) rides the Scalar/ACT HWDGE ring
    concurrently, so head weights arrive early without delaying the stream
    tail."""
    n_groups = n_full + (1 if g_last else 0)
    g_of = lambda t: 8 if t < n_full else g_last
    K_cols = 8 * n_full + g_last  # total 128-row subtiles
    DR = mybir.MatmulPerfMode.DoubleRow

    # stream chunk boundaries in subtile units: one chunk per group. Fine
    # granularity keeps the tail exposure small (only the last small chunk
    # gates the final pooling) while ~0.8us/instruction descriptor-gen still
    # stays well ahead of the ~1.9us/group transfer time.
    if taper:
        TAPS = {12: [6, 4, 2], 10: [4, 4, 2], 8: [4, 2, 2], 6: [4, 2],
                4: [2, 2], 2: [2]}
        bnds = [0]
        while K_cols - bnds[-1] > 12:
            bnds.append(bnds[-1] + 8)
        for step in TAPS.get(K_cols - bnds[-1], [K_cols - bnds[-1]]):
            bnds.append(bnds[-1] + step)
    else:
        bnds = [0]
        while bnds[-1] < K_cols:
            bnds.append(min(bnds[-1] + 8, K_cols))
    n_chunks = len(bnds) - 1
    chunk_of_sub = {}
    for i in range(n_chunks):
        for s in range(bnds[i], bnds[i + 1]):
            chunk_of_sub[s] = i

    nc = bacc.Bacc(enable_partition_id=False)
    hs_d = nc.declare_dram_parameter("hs", [P8, K_cols * H], F8, isOutput=False)
    member_d = nc.declare_dram_parameter(
        "member", [P8, K_cols * LOCAL_B], F8, isOutput=False
    )
    dwT_d = nc.declare_dram_parameter("dwT", [128, 6 * H], F16, isOutput=False)
    cwT_d = nc.declare_dram_parameter("cwT", [128, 6 * T_OUT], F16, isOutput=False)
    ident2_d = nc.declare_dram_parameter("ident2", [32, 32], F16, isOutput=False)
    # smalls blob [128, 8] f32: col 0 invl (rows 0-31), cols 2-7 dense_b chunks
    smalls_d = nc.declare_dram_parameter("smalls", [128, 8], F32, isOutput=False)
    clsb_d = nc.declare_dram_parameter("clsb", [LOCAL_B, T_OUT], F32, isOutput=False)
    out_d = nc.declare_dram_parameter("out", [LOCAL_B, T_OUT], F32, isOutput=True)

    with ExitStack() as ctx:
        member_t = ctx.enter_context(
            nc.sbuf_tensor([P8, K_cols, LOCAL_B], F8)
        )
        sdata = ctx.enter_context(nc.sbuf_tensor([P8, K_cols, H], F8))
        smalls_t = ctx.enter_context(nc.sbuf_tensor([128, 8], F32))
        ident2_t = ctx.enter_context(nc.sbuf_tensor([32, 32], F16))
        dwT_t = ctx.enter_context(nc.sbuf_tensor([128, 6 * H], F16))
        cwT_t = ctx.enter_context(nc.sbuf_tensor([128, 6 * T_OUT], F16))
        pooled_sb = ctx.enter_context(nc.sbuf_tensor([LOCAL_B, H], F16))
        pooledT_sb = ctx.enter_context(nc.sbuf_tensor([128, 6 * LOCAL_B], F16))
        hT_sb = ctx.enter_context(nc.sbuf_tensor([128, 6 * LOCAL_B], F16))
        clsb_t = ctx.enter_context(nc.sbuf_tensor([LOCAL_B, T_OUT], F32))
        logits_sb = ctx.enter_context(nc.sbuf_tensor([LOCAL_B, T_OUT], F32))

        pooled_a = ctx.enter_context(nc.psum_tensor([LOCAL_B, 512], F32))
        pooled_b = ctx.enter_context(nc.psum_tensor([LOCAL_B, 512], F32))
        tp = [
            ctx.enter_context(nc.psum_tensor(f"tp{i}", [128, 512], F16))
            for i in range(3)
        ]
        hps = [
            ctx.enter_context(nc.psum_tensor(f"hps{i}", [128, 512], F32))
            for i in range(2)
        ]
        lps = ctx.enter_context(nc.psum_tensor([LOCAL_B, 512], F32))

        invl_ap = smalls_t[:LOCAL_B, 0:1]
        db6_ap = smalls_t[:, 2:8]

        s_member = nc.alloc_semaphore("s_member")
        if pairsem:
            # chunks 0-5 share sems pairwise (full-count waits), last 3 single
            n_paired = max(0, min(6, n_chunks - 3)) // 2 * 2
            chunk_sem_of = {}
            chunk_tgt_of = {}
            sems = []
            for i in range(n_chunks):
                if i < n_paired:
                    if i % 2 == 0:
                        sems.append(nc.alloc_semaphore(f"s_chunk{len(sems)}"))
                    chunk_sem_of[i] = len(sems) - 1
                    chunk_tgt_of[i] = 32
                else:
                    sems.append(nc.alloc_semaphore(f"s_chunk{len(sems)}"))
                    chunk_sem_of[i] = len(sems) - 1
                    chunk_tgt_of[i] = 16
            s_chunk = sems
        else:
            s_chunk = [nc.alloc_semaphore(f"s_chunk{i}") for i in range(n_chunks)]
            chunk_sem_of = {i: i for i in range(n_chunks)}
            chunk_tgt_of = {i: 16 for i in range(n_chunks)}
        s_smalls = nc.alloc_semaphore("s_smalls")
        s_hw = nc.alloc_semaphore("s_hw")  # dwT+cwT (adjacent on one ring:
        # a full-count wait of 32 implies both transfers complete)
        s_pool = nc.alloc_semaphore("s_pool")
        s_scA = nc.alloc_semaphore("s_scA")
        s_scB = nc.alloc_semaphore("s_scB")
        s_tr = nc.alloc_semaphore("s_tr")
        s_ptcopy = nc.alloc_semaphore("s_ptcopy")
        s_head = nc.alloc_semaphore("s_head")
        s_tanh = nc.alloc_semaphore("s_tanh")
        s_cls = nc.alloc_semaphore("s_cls")
        s_log = nc.alloc_semaphore("s_log")
        s_out = nc.alloc_semaphore("s_out")

        with nc.Block() as block:

            @block.sync
            def _(sync):
                for i in range(n_chunks):
                    a, b = bnds[i], bnds[i + 1]
                    sync.dma_start(
                        out=sdata[:, a:b, :], in_=hs_d[:, a * H : b * H]
                    ).then_inc(s_chunk[chunk_sem_of[i]], 16)
                # Head weights ride the sync queue AFTER every stream chunk:
                # the queue drains FIFO per engine, so they never steal
                # bandwidth mid-stream (a mid-stream transfer delays a chunk
                # by ~4.5us, idles the PE past the HAM window, and the
                # re-throttled PE then runs the rest of the pooling at half
                # clock). They land ~3us after the last chunk, just before
                # the dense layer needs them.
                sync.dma_start(out=dwT_t[:], in_=dwT_d[:]).then_inc(s_hw, 16)
                sync.dma_start(out=cwT_t[:], in_=cwT_d[:]).then_inc(s_hw, 16)
                sync.wait_ge(s_log, 1)
                sync.dma_start(out=out_d[:], in_=logits_sb[:]).then_inc(s_out, 16)
                sync.wait_ge(s_out, 16)

            @block.scalar
            def _(scalar):
                # second HWDGE ring: member + small consts + head weights
                scalar.dma_start(out=member_t[:], in_=member_d[:]).then_inc(
                    s_member, 16
                )
                scalar.dma_start(out=smalls_t[:], in_=smalls_d[:]).then_inc(
                    s_smalls, 16
                )
                scalar.dma_start(out=ident2_t[:], in_=ident2_d[:]).then_inc(
                    s_smalls, 16
                )
                scalar.dma_start(out=clsb_t[:], in_=clsb_d[:]).then_inc(s_smalls, 16)
                scalar.wait_ge(s_smalls, 48)
                # dummy tanh: pulls the ~1.3us ACT_TABLE_LOAD off the critical
                # path. Overwritten by the real jg=0 tanh later.
                nc.scalar.activation(
                    hT_sb[:, 0:1], smalls_t[:, 0:1],
                    mybir.ActivationFunctionType.Tanh,
                ).then_inc(s_tanh, 1)
                # pooled scale, ACT side (concurrent with the DVE side)
                scalar.wait_ge(s_pool, 1)
                if sc3:
                    nc.scalar.activation(
                        pooled_sb[:, 256:512], pooled_a[:, 256:512],
                        mybir.ActivationFunctionType.Copy,
                        scale=invl_ap,
                    ).then_inc(s_scB, 1)
                nc.scalar.activation(
                    pooled_sb[:, 512:H], pooled_b[:, : H - 512],
                    mybir.ActivationFunctionType.Copy,
                    scale=invl_ap,
                ).then_inc(s_scB, 1)
                for jg in range(6):
                    scalar.wait_ge(s_head, jg + 1)
                    nc.scalar.activation(
                        hT_sb[:, jg * LOCAL_B : (jg + 1) * LOCAL_B],
                        hps[jg % 2][:, :LOCAL_B],
                        mybir.ActivationFunctionType.Tanh,
                        bias=db6_ap[:, jg : jg + 1],
                    ).then_inc(s_tanh, 1)

            @block.tensor
            def _(tensor):
                # HAM keep-warm fillers: matmuls on (possibly uninitialized)
                # stream bytes into the hps scratch bank; any NaNs land in
                # PSUM that the dense phase later resets with start=True.
                def filler(n):
                    for _ in range(n):
                        nc.tensor.matmul(
                            hps[0][:, :512],
                            sdata[:, 0:1, 0:128],
                            sdata[:, 0:1, 0:512],
                            start=True, stop=True,
                        )

                filler(12)
                tensor.wait_ge(s_member, 16)
                n_pairs = K_cols // 2
                i_mm = 0
                last_mm = None
                cur_chunk = -1
                for t in range(n_groups):
                    g = g_of(t)
                    k_off = 8 * t
                    for qp in range(0, g, 2):
                        need = chunk_of_sub[k_off + qp + 1]
                        if need > cur_chunk:
                            filler(2)
                            tensor.wait_ge(
                                s_chunk[chunk_sem_of[need]], chunk_tgt_of[need]
                            )
                            cur_chunk = need
                        first, last = i_mm == 0, i_mm == n_pairs - 1
                        if first:
                            # start=True zeroes PSUM at region (bank)
                            # granularity: open each bank with exactly ONE
                            # full-width start=True write (plain fp8 matmuls
                            # over the first subtile pair); everything after
                            # accumulates with start=False.
                            for sub in (0, 1):
                                nc.tensor.matmul(
                                    pooled_a[:, 0:512],
                                    member_t[:, k_off + sub : k_off + sub + 1, :],
                                    sdata[:, k_off + sub : k_off + sub + 1, 0:512],
                                    start=(sub == 0), stop=False,
                                )
                                last_mm = nc.tensor.matmul(
                                    pooled_b[:, 0:256],
                                    member_t[:, k_off + sub : k_off + sub + 1, :],
                                    sdata[:, k_off + sub : k_off + sub + 1, 512:768],
                                    start=(sub == 0), stop=last,
                                )
                        else:
                            lhsT = member_t[:, k_off + qp : k_off + qp + 2, :]
                            pair = sdata[:, k_off + qp : k_off + qp + 2, :]
                            if wide:
                                # rhs free 1024 (fp8 moving max per HW, above
                                # bass's unused fp32-era 512 constant): 2
                                # matmuls per pair, bank A one full region
                                nc.tensor.matmul(
                                    pooled_a[:, 0:512], lhsT, pair[:, :, 0:512],
                                    start=False, stop=last, perf_mode=DR,
                                )
                                last_mm = nc.tensor.matmul(
                                    pooled_b[:, 0:256], lhsT, pair[:, :, 512:768],
                                    start=False, stop=last, perf_mode=DR,
                                )
                            else:
                                for out_ap, h0 in (
                                    (pooled_a[:, 0:256], 0),
                                    (pooled_a[:, 256:512], 256),
                                    (pooled_b[:, 0:256], 512),
                                ):
                                    last_mm = nc.tensor.matmul(
                                        out_ap, lhsT,
                                        pair[:, :, h0 : h0 + 256],
                                        start=False, stop=last, perf_mode=DR,
                                    )
                        i_mm += 1
                last_mm.then_inc(s_pool, 1)
                # transposes (need the scales + the identity matrix)
                filler(2)
                tensor.wait_ge(s_smalls, 48)
                for c in range(6):
                    if sc3:
                        if c < 2:
                            tensor.wait_ge(s_scA, 1)
                        else:
                            tensor.wait_ge(s_scB, 1 if c < 4 else 2)
                    else:
                        tensor.wait_ge(s_scA if c < 4 else s_scB, 1)
                    if c >= 3:
                        tensor.wait_ge(s_ptcopy, c - 2)
                    nc.tensor.transpose(
                        tp[c % 3][:, :LOCAL_B],
                        pooled_sb[:, c * 128 : (c + 1) * 128],
                        ident2_t[:],
                    ).then_inc(s_tr, 1)
                # dense layer
                filler(2)
                tensor.wait_ge(s_ptcopy, 6)
                tensor.wait_ge(s_hw, 32)
                for jg in range(6):
                    if jg >= 2:
                        tensor.wait_ge(s_tanh, jg)
                    for c in range(6):
                        mm = nc.tensor.matmul(
                            hps[jg % 2][:, :LOCAL_B],
                            dwT_t[:, c * H + jg * 128 : c * H + (jg + 1) * 128],
                            pooledT_sb[:, c * LOCAL_B : (c + 1) * LOCAL_B],
                            start=(c == 0), stop=(c == 5),
                        )
                    mm.then_inc(s_head, 1)
                # classifier: logits[b, t] -- hT chunk is the stationary
                # operand so the output lands batch-major.
                for jg in range(6):
                    tensor.wait_ge(s_tanh, jg + 2)
                    mm = nc.tensor.matmul(
                        lps[:, :T_OUT],
                        hT_sb[:, jg * LOCAL_B : (jg + 1) * LOCAL_B],
                        cwT_t[:, jg * T_OUT : (jg + 1) * T_OUT],
                        start=(jg == 0), stop=(jg == 5),
                    )
                mm.then_inc(s_cls, 1)

            @block.vector
            def _(vector):
                vector.wait_ge(s_smalls, 48)
                vector.wait_ge(s_pool, 1)
                nc.vector.tensor_scalar_mul(
                    pooled_sb[:, 0 : 256 if sc3 else 512],
                    pooled_a[:, 0 : 256 if sc3 else 512], invl_ap
                ).then_inc(s_scA, 1)
                for c in range(6):
                    vector.wait_ge(s_tr, c + 1)
                    nc.vector.tensor_copy(
                        pooledT_sb[:, c * LOCAL_B : (c + 1) * LOCAL_B],
                        tp[c % 3][:, :LOCAL_B],
                    ).then_inc(s_ptcopy, 1)
                vector.wait_ge(s_cls, 1)
                nc.vector.tensor_add(
                    logits_sb[:], lps[:, :T_OUT], clsb_t[:]
                ).then_inc(s_log, 1)

    nc.compile()
    return nc


def _quantize_feedback_f8(hs, lens):
    """e4m3 quantization with per-(sample, channel) error feedback along the
    row sequence, plus a final mop-up row holding the residual. The device sums
    rows, so per-row quantization errors telescope: the pooled sum sees only
    the (quantized) final residual."""
    f32 = np.float32
    q = np.zeros((B, S, H), NP_F8)
    err = np.zeros((B, H), f32)
    maxlen = int(lens.max())
    hsf = hs.astype(f32, copy=False)
    for s in range(1, maxlen + 1):
        active = lens >= s
        x = hsf[active, s, :] + err[active]
        qs = x.astype(NP_F8)
        q[active, s, :] = qs
        err[active] = x - qs.astype(f32)
    mop = err.astype(NP_F8)
    return q, mop


def _kernel_f8(hs, lens, dense_w, dense_b, cls_w, cls_b):
    global last_results
    # ---- assign samples to cores: greedy LPT on (len+1) with a 32-per-core cap
    w = lens + 1  # +1 for the mop-up row
    order = np.argsort(-w, kind="stable")
    core_samples = [[] for _ in range(N_CORES)]
    load = np.zeros(N_CORES, dtype=np.int64)
    for b in order:
        open_cores = [c for c in range(N_CORES) if len(core_samples[c]) < LOCAL_B]
        c = min(open_cores, key=lambda c: load[c])
        core_samples[c].append(int(b))
        load[c] += int(w[b])
    max_rows = int(load.max())
    n_full, rem = divmod(max_rows, 8 * P8)
    g_last = -(-rem // P8)
    g_last += g_last % 2  # DoubleRow consumes subtile pairs
    if g_last == 8:
        n_full, g_last = n_full + 1, 0
    n_groups = n_full + (1 if g_last else 0)
    NR = 8 * P8 * n_full + P8 * g_last
    K_cols = 8 * n_full + g_last

    key = ("f8", n_full, g_last, WIDE, TAPER, SC3, PAIRSEM)
    if key not in _cache:
        _cache[key] = _build_program_f8(
            n_full, g_last, bool(WIDE), bool(TAPER), bool(SC3), bool(PAIRSEM)
        )
    nc = _cache[key]

    q, mop = _quantize_feedback_f8(hs, lens)

    # ---- shared (replicated) head tensors, fp16
    dwT_host = np.empty((128, 6 * H), np.float32)
    for c in range(6):
        dwT_host[:, c * H : (c + 1) * H] = dense_w[:, c * 128 : (c + 1) * 128].T
    cwT_host = np.empty((128, 6 * T_OUT), np.float32)
    for jg in range(6):
        cwT_host[:, jg * T_OUT : (jg + 1) * T_OUT] = cls_w[:, jg * 128 : (jg + 1) * 128].T
    dwT_host = dwT_host.astype(np.float16)
    cwT_host = cwT_host.astype(np.float16)
    ident2_host = np.eye(32, dtype=np.float16)
    db6_host = np.ascontiguousarray(dense_b.reshape(6, 128).T)
    clsb_host = np.ascontiguousarray(
        np.broadcast_to(cls_b, (LOCAL_B, T_OUT)).astype(np.float32)
    )

    q2 = q.reshape(B * S, H)
    in_maps = []
    for c in range(N_CORES):
        samples = core_samples[c]
        lens_c = lens[samples]
        packed = np.zeros((NR, H), NP_F8)
        pos = 0
        for b in samples:
            L = int(lens[b])
            packed[pos : pos + L] = q2[b * S + 1 : b * S + 1 + L]
            packed[pos + L] = mop[b]
            pos += L + 1

        # membership: row j -> (group t, partition p, subtile q); G_t rows are
        # consecutive per partition within a group.
        j = np.arange(pos)
        t = np.minimum(j // (8 * P8), n_groups - 1)
        j2 = j - t * (8 * P8)
        g_t = np.where(t < n_full, 8, g_last)
        p = j2 // g_t
        qsub = j2 - p * g_t
        kcol = 8 * t + qsub
        local_b = np.repeat(np.arange(LOCAL_B), lens_c + 1)
        mem = np.zeros((P8, K_cols * LOCAL_B), NP_F8)
        mem[p, kcol * LOCAL_B + local_b] = NP_F8(1.0)

        smalls = np.zeros((128, 8), np.float32)
        smalls[:LOCAL_B, 0] = 1.0 / lens_c.astype(np.float32)
        smalls[:, 2:8] = db6_host
        im = {
            "member": mem,
            "dwT": dwT_host,
            "cwT": cwT_host,
            "ident2": ident2_host,
            "smalls": smalls,
            "clsb": clsb_host,
        }
        # sdata layout [P8, K_cols, H]: row j -> sdata[p_j, kcol_j, :]
        arr = np.zeros((P8, K_cols, H), NP_F8)
        arr[p, kcol] = packed[:pos]
        im["hs"] = arr.reshape(P8, K_cols * H)
        in_maps.append(im)

    trace = bool(os.environ.get("KERNEL_TRACE"))
    try:
        res = bass_utils.run_bass_kernel_spmd(
            nc, in_maps, list(range(N_CORES)), trace=trace
        )
    except Exception:
        res = bass_utils.run_bass_kernel_spmd(
            nc, in_maps, list(range(N_CORES)), trace=trace
        )
    last_results = res

    logits = np.zeros((B, T_OUT), np.float32)
    for c in range(N_CORES):
        logits[core_samples[c], :] = res.results[c]["out"]
    return logits


def _build_program_raw(T_g: int, mode: str, p_last: int = 128) -> bass.Bass:
    """Hand-synchronized variant: one FIFO HWDGE ring delivers member, the
    stream tiles (in consumption order), then the head weights; each engine's
    program carries explicit sem waits. PSUM is budgeted bank-by-bank:
    pooled_a, pooled_b, tp0-2, hps0-1, lps = 8 banks."""
    sdt = mybir.dt.float16 if mode == "f16" else mybir.dt.bfloat16
    n_streams = 2 if mode == "f32x2" else 1
    # Head dtype: fp16/bf16 single-stream modes run the whole head in the
    # stream dtype (fp32 head matmuls cost 2 LDWEIGHTS+MATMUL passes each —
    # measured ~17us for the 48 head matmuls vs ~5us in fp16). The f32x2
    # accuracy mode keeps the head in fp32.
    hdt = F32 if n_streams == 2 else sdt
    W = G * H

    # No collectives -> no partition id; skipping it drops 5 per-engine
    # TENSOR_LOADs (~2us) from the launch preamble.
    nc = bacc.Bacc(enable_partition_id=False)
    streams_d = [
        nc.declare_dram_parameter(f"hs{i}", [T_g, 128, W], sdt, isOutput=False)
        for i in range(n_streams)
    ]
    member_d = nc.declare_dram_parameter(
        "member", [128, T_g * G * LOCAL_B], sdt, isOutput=False
    )
    dwT_d = nc.declare_dram_parameter("dwT", [128, 6 * H], hdt, isOutput=False)
    cwT_d = nc.declare_dram_parameter("cwT", [128, 6 * T_OUT], hdt, isOutput=False)
    ident2_d = nc.declare_dram_parameter("ident2", [32, 32], hdt, isOutput=False)
    # smalls blob [128, 40] f32: col 0 invl (rows 0-31), col 1 cls_b (rows
    # 0-95), cols 2-7 dense_b chunks, cols 8-39 identity (rows 0-31).
    smalls_d = nc.declare_dram_parameter("smalls", [128, 40], F32, isOutput=False)
    # cls_b pre-broadcast to [32, 96] on the host: lets the classifier output
    # land as logits [b, t] (32 descriptors x 384B on the store instead of 96
    # x 128B — the store's tail rides the slowest SDMA engine).
    clsb_d = nc.declare_dram_parameter("clsb", [LOCAL_B, T_OUT], F32, isOutput=False)
    out_d = nc.declare_dram_parameter("out", [LOCAL_B, T_OUT], F32, isOutput=True)

    with ExitStack() as ctx:
        member_t = ctx.enter_context(
            nc.sbuf_tensor([128, T_g * G * LOCAL_B], sdt)
        )
        stile = [
            [
                ctx.enter_context(nc.sbuf_tensor(f"stile{si}_{t}", [128, W], sdt))
                for t in range(T_g)
            ]
            for si in range(n_streams)
        ]
        smalls_t = ctx.enter_context(nc.sbuf_tensor([128, 40], F32))
        ident2_t = ctx.enter_context(nc.sbuf_tensor([32, 32], hdt))
        dwT_t = ctx.enter_context(nc.sbuf_tensor([128, 6 * H], hdt))
        cwT_t = ctx.enter_context(nc.sbuf_tensor([128, 6 * T_OUT], hdt))
        pooled_sb = ctx.enter_context(nc.sbuf_tensor([LOCAL_B, H], hdt))
        pooledT_sb = ctx.enter_context(nc.sbuf_tensor([128, 6 * LOCAL_B], hdt))
        hT_sb = ctx.enter_context(nc.sbuf_tensor([128, 6 * LOCAL_B], hdt))
        clsb_t = ctx.enter_context(nc.sbuf_tensor([LOCAL_B, T_OUT], F32))
        logits_sb = ctx.enter_context(nc.sbuf_tensor([LOCAL_B, T_OUT], F32))
        warm_sb = ctx.enter_context(nc.sbuf_tensor([128, 512], sdt))

        pooled_a = ctx.enter_context(nc.psum_tensor([LOCAL_B, 512], F32))
        pooled_b = ctx.enter_context(nc.psum_tensor([LOCAL_B, 512], F32))
        tp = [
            ctx.enter_context(nc.psum_tensor(f"tp{i}", [128, 512], hdt))
            for i in range(3)
        ]
        hps = [
            ctx.enter_context(nc.psum_tensor(f"hps{i}", [128, 512], F32))
            for i in range(2)
        ]
        lps = ctx.enter_context(nc.psum_tensor([LOCAL_B, 512], F32))

        invl_ap = smalls_t[:LOCAL_B, 0:1]
        db6_ap = smalls_t[:, 2:8]

        # Single-stream modes: each stream tile arrives as two half-DMAs with
        # their own sems. The matmuls for the first half run while the second
        # half transfers — and when a slow SDMA engine dribbles the ring
        # tail, only the last half-tile's 8 matmuls wait on it.
        halved = n_streams == 1
        s_member = nc.alloc_semaphore("s_member")
        s_stream = [nc.alloc_semaphore(f"s_stream{t}") for t in range(T_g)]
        s_streamB = [nc.alloc_semaphore(f"s_streamB{t}") for t in range(T_g)]
        s_smalls = nc.alloc_semaphore("s_smalls")
        s_hw = nc.alloc_semaphore("s_hw")  # dwT+cwT (adjacent on one ring:
        # a full-count wait of 32 implies both transfers complete)
        s_pool = nc.alloc_semaphore("s_pool")
        s_scaled = nc.alloc_semaphore("s_scaled")
        s_tr = nc.alloc_semaphore("s_tr")
        s_ptcopy = nc.alloc_semaphore("s_ptcopy")
        s_head = nc.alloc_semaphore("s_head")
        s_tanh = nc.alloc_semaphore("s_tanh")
        s_cls = nc.alloc_semaphore("s_cls")
        s_log = nc.alloc_semaphore("s_log")
        s_out = nc.alloc_semaphore("s_out")
        s_warm = nc.alloc_semaphore("s_warm")

        with nc.Block() as block:

            @block.gpsimd
            def _(gpsimd):
                nc.gpsimd.memset(warm_sb[:], 0.0).then_inc(s_warm, 1)
                # The last group holds only the load-balance remainder: under
                # the p-major packing its real rows occupy partitions
                # [0, p_last). Those above are never transferred (the DMA
                # below skips them) — zero once so the matmuls read 0s
                # (membership is 0 there, but fp16 garbage could be NaN).
                if p_last < 128:
                    for si in range(n_streams):
                        nc.gpsimd.memset(
                            stile[si][T_g - 1][p_last:, :], 0.0
                        ).then_inc(s_warm, 1)

            @block.sync
            def _(sync):
                # FIFO ring: group-0 slice of member first, then stream tile
                # 0, then the rest in consumption order — the first matmul
                # needs only the first two transfers (~1.6MB).
                g0 = G * LOCAL_B
                sync.dma_start(
                    out=member_t[:, :g0], in_=member_d[:, :g0]
                ).then_inc(s_member, 16)
                def stream_tile(t):
                    pl = p_last if t == T_g - 1 else 128
                    if halved:
                        hw2 = W // 2
                        sync.dma_start(
                            out=stile[0][t][:pl, :hw2],
                            in_=streams_d[0][t][:pl, :hw2],
                        ).then_inc(s_stream[t], 16)
                        sync.dma_start(
                            out=stile[0][t][:pl, hw2:],
                            in_=streams_d[0][t][:pl, hw2:],
                        ).then_inc(s_streamB[t], 16)
                    else:
                        for si in range(n_streams):
                            sync.dma_start(
                                out=stile[si][t][:pl, :], in_=streams_d[si][t][:pl, :]
                            ).then_inc(s_stream[t], 16)

                stream_tile(0)
                sync.dma_start(
                    out=member_t[:, g0:], in_=member_d[:, g0:]
                ).then_inc(s_member1, 16)
                for t in range(1, T_g):
                    stream_tile(t)
                    if t == 2:
                        # tiny consts ride mid-ring: early enough that their
                        # last descriptors never trail on the slowest SDMA
                        # engine (which stalls the head), late enough that
                        # their issue slots don't delay the ramp-phase
                        # transfers the first matmuls wait on.
                        sync.dma_start(
                            out=smalls_t[:], in_=smalls_d[:]
                        ).then_inc(s_smalls, 16)
                        sync.dma_start(
                            out=ident2_t[:], in_=ident2_d[:]
                        ).then_inc(s_smalls, 16)
                        sync.dma_start(out=clsb_t[:], in_=clsb_d[:]).then_inc(
                            s_smalls, 16
                        )
                sync.dma_start(out=dwT_t[:], in_=dwT_d[:]).then_inc(s_hw, 16)
                sync.dma_start(out=cwT_t[:], in_=cwT_d[:]).then_inc(s_hw, 16)
                # output store (waits for the DVE bias-add)
                sync.wait_ge(s_log, 1)
                sync.dma_start(out=out_d[:], in_=logits_sb[:]).then_inc(s_out, 16)
                sync.wait_ge(s_out, 16)

            @block.tensor
            def _(tensor):
                # HAM management: the PE clock-gate re-throttles to 4/8 after
                # an idle activity window (~3.4us). The kernel is DMA-paced,
                # so the PE would idle ~1us per group — enough, with bad
                # window phase, to oscillate between 1.2 and 2.4 GHz. Filler
                # matmuls on a zeroed scratch tile (into the hps bank, which
                # the dense phase later resets with start=True) keep the PE
                # busy through every wait.
                def filler(n):
                    for _ in range(n):
                        nc.tensor.matmul(
                            hps[0][:, :512],
                            warm_sb[:, :128],
                            warm_sb[:, :512],
                            start=True, stop=True,
                        )

                tensor.wait_ge(s_warm, 1 + (n_streams if p_last < 128 else 0))
                filler(12)
                n_mm = T_g * G * n_streams
                i_mm = 0
                last_mm = None
                for t in range(T_g):
                    if t == 0:
                        tensor.wait_ge(s_member, 16)
                    else:
                        filler(4)
                        tensor.wait_ge(s_member1, 16)
                    for q in range(G):
                        if q == 0:
                            tensor.wait_ge(s_stream[t], 16 * n_streams)
                        elif halved and q == G // 2:
                            tensor.wait_ge(s_streamB[t], 16)
                        k = t * G + q
                        lhsT = member_t[:, k * LOCAL_B : (k + 1) * LOCAL_B]
                        for si in range(n_streams):
                            st = stile[si][t]
                            first, last = i_mm == 0, i_mm == n_mm - 1
                            nc.tensor.matmul(
                                pooled_a[:, :512], lhsT,
                                st[:, q * H : q * H + 512],
                                start=first, stop=last,
                            )
                            last_mm = nc.tensor.matmul(
                                pooled_b[:, : H - 512], lhsT,
                                st[:, q * H + 512 : (q + 1) * H],
                                start=first, stop=last,
                            )
                            i_mm += 1
                last_mm.then_inc(s_pool, 1)
                # transposes (need DVE scale + the identity matrix)
                filler(3)
                tensor.wait_ge(s_smalls, 48)
                for c in range(6):
                    tensor.wait_ge(s_scaled, 1 if c < 4 else 2)
                    if c >= 3:
                        tensor.wait_ge(s_ptcopy, c - 2)
                    nc.tensor.transpose(
                        tp[c % 3][:, :LOCAL_B],
                        pooled_sb[:, c * 128 : (c + 1) * 128],
                        ident2_t[:],
                    ).then_inc(s_tr, 1)
                # dense layer
                filler(2)
                tensor.wait_ge(s_ptcopy, 6)
                tensor.wait_ge(s_hw, 32)
                for jg in range(6):
                    if jg >= 2:
                        tensor.wait_ge(s_tanh, jg - 1)
                    for c in range(6):
                        mm = nc.tensor.matmul(
                            hps[jg % 2][:, :LOCAL_B],
                            dwT_t[:, c * H + jg * 128 : c * H + (jg + 1) * 128],
                            pooledT_sb[:, c * LOCAL_B : (c + 1) * LOCAL_B],
                            start=(c == 0), stop=(c == 5),
                        )
                    mm.then_inc(s_head, 1)
                # classifier
                # classifier: logits[b, t] — hT chunk is the stationary
                # operand so the output lands batch-major.
                tensor.wait_ge(s_cwT, 16)
                for jg in range(6):
                    tensor.wait_ge(s_tanh, jg + 1)
                    mm = nc.tensor.matmul(
                        lps[:, :T_OUT],
                        hT_sb[:, jg * LOCAL_B : (jg + 1) * LOCAL_B],
                        cwT_t[:, jg * T_OUT : (jg + 1) * T_OUT],
                        start=(jg == 0), stop=(jg == 5),
                    )
                mm.then_inc(s_cls, 1)

            @block.vector
            def _(vector):
                vector.wait_ge(s_smalls, 48)
                vector.wait_ge(s_pool, 1)
                nc.vector.tensor_scalar_mul(
                    pooled_sb[:, 0:512], pooled_a[:, :512], invl_ap
                ).then_inc(s_scaled, 1)
                nc.vector.tensor_scalar_mul(
                    pooled_sb[:, 512:H], pooled_b[:, : H - 512], invl_ap
                ).then_inc(s_scaled, 1)
                for c in range(6):
                    vector.wait_ge(s_tr, c + 1)
                    nc.vector.tensor_copy(
                        pooledT_sb[:, c * LOCAL_B : (c + 1) * LOCAL_B],
                        tp[c % 3][:, :LOCAL_B],
                    ).then_inc(s_ptcopy, 1)
                vector.wait_ge(s_cls, 1)
                nc.vector.tensor_add(
                    logits_sb[:], lps[:, :T_OUT], clsb_t[:]
                ).then_inc(s_log, 1)

            @block.scalar
            def _(scalar):
                scalar.wait_ge(s_smalls, 48)
                for jg in range(6):
                    scalar.wait_ge(s_head, jg + 1)
                    nc.scalar.activation(
                        hT_sb[:, jg * LOCAL_B : (jg + 1) * LOCAL_B],
                        hps[jg % 2][:, :LOCAL_B],
                        mybir.ActivationFunctionType.Tanh,
                        bias=db6_ap[:, jg : jg + 1],
                    ).then_inc(s_tanh, 1)

    nc.compile()
    return nc


def kernel(hidden_states, pivot_len_list, dense_w, dense_b, cls_w, cls_b):
    global last_results
    hs = np.ascontiguousarray(np.asarray(hidden_states, dtype=np.float32))
    lens = np.asarray(pivot_len_list).astype(np.int64)
    dense_w = np.asarray(dense_w, dtype=np.float32)
    dense_b = np.asarray(dense_b, dtype=np.float32)
    cls_w = np.asarray(cls_w, dtype=np.float32)
    cls_b = np.asarray(cls_b, dtype=np.float32)
    assert hs.shape == (B, S, H), hs.shape
    assert lens.shape == (B,), lens.shape

    if MODE == "f8":
        return _kernel_f8(hs, lens, dense_w, dense_b, cls_w, cls_b)

    mode = MODE
    np_sdt = np.float16 if mode == "f16" else ml_dtypes.bfloat16

    # ---- assign samples to cores: greedy LPT with a hard 32-per-core cap
    order = np.argsort(-lens, kind="stable")
    core_samples = [[] for _ in range(N_CORES)]
    load = np.zeros(N_CORES, dtype=np.int64)
    for b in order:
        open_cores = [c for c in range(N_CORES) if len(core_samples[c]) < LOCAL_B]
        c = min(open_cores, key=lambda c: load[c])
        core_samples[c].append(int(b))
        load[c] += int(lens[b])
    T_g = max(1, -(-int(load.max()) // ROWS_PER_GROUP))

    impl = IMPL
    # Partitions actually occupied in the final (remainder) group under the
    # p-major packing; the rest of that tile is padding and never transferred.
    rows_last = int(load.max()) - (T_g - 1) * ROWS_PER_GROUP
    # partition offsets must be 32-aligned (engine base-partition constraint)
    p_last = min(128, max(32, 32 * -(--(-rows_last // G) // 32)))
    key = (T_g, mode, impl, p_last)
    if key not in _cache:
        if impl == "raw":
            _cache[key] = _build_program_raw(T_g, mode, p_last)
        else:
            _cache[key] = _build_program(T_g, mode)
    nc = _cache[key]

    # ---- shared (replicated) head tensors
    dwT_host = np.empty((128, 6 * H), np.float32)
    for c in range(6):
        dwT_host[:, c * H : (c + 1) * H] = dense_w[:, c * 128 : (c + 1) * 128].T
    cwT_host = np.empty((128, 6 * T_OUT), np.float32)
    for jg in range(6):
        cwT_host[:, jg * T_OUT : (jg + 1) * T_OUT] = cls_w[:, jg * 128 : (jg + 1) * 128].T
    db6_host = np.ascontiguousarray(dense_b.reshape(6, 128).T)
    cb1_host = np.ascontiguousarray(cls_b.reshape(T_OUT, 1))
    ident_host = np.eye(32, dtype=np.float32)

    # ---- per-core packing
    hs2 = hs.reshape(B * S, H)
    NR = T_g * ROWS_PER_GROUP
    in_maps = []
    for c in range(N_CORES):
        samples = core_samples[c]
        lens_c = lens[samples]
        idx = np.concatenate(
            [np.arange(b * S + 1, b * S + 1 + lens[b]) for b in samples]
        )
        n = idx.size
        packed = np.zeros((NR, H), np.float32)
        packed[:n] = hs2[idx]
        if mode == "f32x2":
            hi = packed.astype(ml_dtypes.bfloat16)
            lo = (packed - hi.astype(np.float32)).astype(ml_dtypes.bfloat16)
            stream_arrays = [hi, lo]
        else:
            stream_arrays = [packed.astype(np_sdt)]

        j = np.arange(n)
        tt = j // ROWS_PER_GROUP
        p = (j % ROWS_PER_GROUP) // G
        q = j % G
        local_b = np.repeat(np.arange(LOCAL_B), lens_c)
        mem = np.zeros((128, T_g * G * LOCAL_B), np_sdt)
        mem[p, (tt * G + q) * LOCAL_B + local_b] = np_sdt(1.0)

        invl_host = (1.0 / lens_c.astype(np.float32)).reshape(LOCAL_B, 1)
        im = {"member": mem, "dwT": dwT_host, "cwT": cwT_host}
        if impl == "raw":
            np_hdt = np.float32 if mode == "f32x2" else np_sdt
            im["dwT"] = dwT_host.astype(np_hdt)
            im["cwT"] = cwT_host.astype(np_hdt)
            im["ident2"] = np.eye(32, dtype=np.float32).astype(np_hdt)
            im["clsb"] = np.ascontiguousarray(
                np.broadcast_to(cls_b, (LOCAL_B, T_OUT)).astype(np.float32)
            )
            smalls = np.zeros((128, 40), np.float32)
            smalls[:LOCAL_B, 0:1] = invl_host
            smalls[:T_OUT, 1:2] = cb1_host
            smalls[:, 2:8] = db6_host
            smalls[:32, 8:40] = ident_host
            im["smalls"] = smalls
        else:
            im["db6"] = db6_host
            im["cb1"] = cb1_host
            im["invl"] = invl_host
            im["ident"] = ident_host
        for i, arr in enumerate(stream_arrays):
            im[f"hs{i}"] = arr.reshape(T_g, 128, G * H)
        in_maps.append(im)

    trace = bool(os.environ.get("KERNEL_TRACE"))
    try:
        res = bass_utils.run_bass_kernel_spmd(
            nc, in_maps, list(range(N_CORES)), trace=trace
        )
    except Exception:
        # Transient NRT device errors (e.g. NRT_EXEC_UNIT_UNRECOVERABLE after
        # many back-to-back launches) clear on retry.
        res = bass_utils.run_bass_kernel_spmd(
            nc, in_maps, list(range(N_CORES)), trace=trace
        )
    last_results = res

    logits = np.zeros((B, T_OUT), np.float32)
    for c in range(N_CORES):
        o = res.results[c]["out"]
        logits[core_samples[c], :] = o if impl == "raw" else o.T
    return logits

